# revision 25
# baseline (speedup 1.0000x reference)
"""Distributed Trainium2 kernel for GQA attention (B=2, S=2048, D=4096,
32 q-heads / 8 kv-heads, HD=128, RoPE, additive causal mask) on 8
NeuronCores.

Sharding: tensor-parallel over heads (4 q-heads + 1 kv-head per core).

Phase 1 computes the QKV projections weights-stationary so q^T/k^T come
out directly in [feat, tok] layout (RoPE applied in that layout via a
partition pair-swap stream shuffle); v is rotated back to [tok, feat]
with PE transposes. Phase 2 computes scores transposed (k stationary,
q^T moving -> s^T tiles), uses a max-free softmax (exp straight out of
PSUM, causal mask added only on diagonal tiles), feeds e^T directly to
the pv matmul (no p transposes), and accumulates per-row softmax
denominators with a ones-stationary matmul. Unnormalized o^T plus the
f32 row sums ride two AllToAlls (after head pair 0/1 and 2/3). Phase 3
normalizes the received o^T by the broadcast reciprocal row sums and
runs the Wo GEMM for this core's 512-token slice, ordering the
contraction so the second AllToAll hides under the first half's
compute. All matmuls bf16 with f32 PSUM accumulation.
"""
import sys

sys.path.insert(0, "/opt/trn_rl_repo")

import math
from contextlib import ExitStack
import numpy as np
import ml_dtypes

import concourse.bass as bass
import concourse.tile as tile
from concourse import bacc, mybir
from concourse.bass_utils import run_bass_kernel_spmd
from concourse.masks import make_identity

F32 = mybir.dt.float32
BF16 = mybir.dt.bfloat16
AF = mybir.ActivationFunctionType
OP = mybir.AluOpType

NCORES = 8
B, S, D = 2, 2048, 4096
NH, NKV, HD = 32, 8, 128
QH = NH // NCORES          # 4 q-heads per core
TOK = B * S                # 4096
TB = TOK // 512            # 8 blocks of 512 tokens
TT = TOK // 128            # 32 token tiles
TSLICE = TOK // NCORES     # 512 tokens out per core
NEG_INF = -1e9
SWAP32 = [i ^ 1 for i in range(32)]   # even/odd partition pair swap


def _build(mask_mode: str):
    nc = bacc.Bacc("TRN2", target_bir_lowering=False, debug=False,
                   enable_asserts=False, num_devices=NCORES)

    xT_e = nc.dram_tensor("xT", [D, TOK], BF16, kind="ExternalInput")
    Wq_e = nc.dram_tensor("Wqkv", [D, (QH + 2) * HD], BF16,
                          kind="ExternalInput")
    Wo_e = nc.dram_tensor("Wo", [D, D], BF16, kind="ExternalInput")
    # RoPE tables in transposed layout [feat 128, pos]: cq/sq q-scaled,
    # sq/sk carry the parity sign (-sin on even, +sin on odd partitions).
    cq_e = nc.dram_tensor("cqT", [128, S], F32, kind="ExternalInput")
    sq_e = nc.dram_tensor("sqT", [128, S], F32, kind="ExternalInput")
    ck_e = nc.dram_tensor("ckT", [128, S], F32, kind="ExternalInput")
    sk_e = nc.dram_tensor("skT", [128, S], F32, kind="ExternalInput")
    if mask_mode == "causal":
        mask_e = nc.dram_tensor("masktri", [128, 128], F32,
                                kind="ExternalInput")
    elif mask_mode == "general":
        mask_e = nc.dram_tensor("maskT", [S, S], F32, kind="ExternalInput")
    else:
        mask_e = None
    bias_e = nc.dram_tensor("biasB", [128, 1], F32, kind="ExternalInput") \
        if mask_mode == "general" else None
    out_e = nc.dram_tensor("out", [TSLICE, D], F32, kind="ExternalOutput")

    with tile.TileContext(nc) as tc, ExitStack() as ctx:
        _body(ctx, tc, mask_mode, xT_e, Wq_e, Wo_e,
              cq_e, sq_e, ck_e, sk_e, mask_e, bias_e, out_e)
    nc.compile()
    return nc


def _body(ctx, tc, mask_mode, xT_e, Wq_e, Wo_e,
          cq_e, sq_e, ck_e, sk_e, mask_e, bias_e, out_e):
    nc = tc.nc
    causal = mask_mode == "causal"

    consts = ctx.enter_context(tc.tile_pool(name="consts", bufs=1))
    ident_b = consts.tile([128, 128], BF16)
    make_identity(nc, ident_b[:])
    ones_sb = consts.tile([128, 128], BF16)
    nc.vector.memset(ones_sb[:], 1.0)

    # persistent across phases
    kv_pool = ctx.enter_context(tc.tile_pool(name="kv", bufs=1))
    qT_sb = kv_pool.tile([128, QH, TB, 512], BF16)  # [hd, h, tb, tok]
    kT_sb = kv_pool.tile([128, TB, 512], BF16)      # [hd, tb, tok]
    v_sb = kv_pool.tile([128, TT, 128], BF16)       # [tok, kt, feat]

    dram = ctx.enter_context(tc.tile_pool(name="dram", bufs=1, space="DRAM"))
    # per slot: 256 rows of o^T (2 heads) + 4 rows = 2x[1,512] f32 rowsums
    a2a_in1 = dram.tile([NCORES, 260, TSLICE], BF16)
    a2a_out1 = dram.tile([NCORES, 260, TSLICE], BF16)
    a2a_in2 = dram.tile([NCORES, 260, TSLICE], BF16)
    a2a_out2 = dram.tile([NCORES, 260, TSLICE], BF16)
    rdram = dram.tile([2, 16, 512], F32)

    # ---------------- Phase 1: QKV projections + RoPE --------------------
    with tc.tile_pool(name="pj_w", bufs=1) as wpool, \
         tc.tile_pool(name="pj_x", bufs=2) as xpool, \
         tc.tile_pool(name="pj_rt", bufs=1) as rtpool, \
         tc.tile_pool(name="pj_qs", bufs=2) as qspool, \
         tc.tile_pool(name="pj_tmp", bufs=1) as tmppool, \
         tc.tile_pool(name="pj_st", bufs=3) as stpool, \
         tc.tile_pool(name="pj_psq", bufs=1, space="PSUM") as psq_pool, \
         tc.tile_pool(name="pj_pskv", bufs=1, space="PSUM") as pskv_pool, \
         tc.tile_pool(name="pj_pstr", bufs=2, space="PSUM") as pstr_pool:

        Wq_sb = wpool.tile([128, 32, (QH + 2) * HD], BF16)
        wq_re = Wq_e.ap().rearrange("(k p) f -> p k f", p=128)
        # split the weight load so the first k-slices land fast
        nc.sync.dma_start(out=Wq_sb[:, 0:1, :], in_=wq_re[:, 0:1, :])
        nc.sync.dma_start(out=Wq_sb[:, 1:4, :], in_=wq_re[:, 1:4, :])
        nc.sync.dma_start(out=Wq_sb[:, 4:16, :], in_=wq_re[:, 4:16, :])
        nc.sync.dma_start(out=Wq_sb[:, 16:32, :], in_=wq_re[:, 16:32, :])

        ctabs = {}
        for nm, te in (("cq", cq_e), ("sq", sq_e), ("ck", ck_e), ("sk", sk_e)):
            t = rtpool.tile([128, 4, 512], F32, name=f"tab_{nm}",
                            tag=f"tab_{nm}")
            nc.scalar.dma_start(
                out=t[:], in_=te.ap().rearrange("p (sb t) -> p sb t", t=512))
            ctabs[nm] = t

        def rep2(ap):  # repeat a [128, 512] table slice 2x along free dim
            return bass.AP(tensor=ap.tensor, offset=ap.offset,
                           ap=[ap.ap[0], [0, 2], ap.ap[-1]])

        for tb in range(TB):
            sblk = tb % 4   # position block within batch
            qps = psq_pool.tile([128, QH, 512], F32)
            kvps = pskv_pool.tile([128, 2, 512], F32)
            for kq in range(4):
                xsl = xpool.tile([128, 8, 512], BF16)
                xsrc = xT_e.ap()[:, tb * 512:(tb + 1) * 512] \
                    .rearrange("(k p) t -> p k t", p=128)
                if tb == 0 and kq == 0:
                    # split the very first load so matmuls start sooner
                    nc.gpsimd.dma_start(out=xsl[:, 0:2, :],
                                        in_=xsrc[:, 0:2, :])
                    nc.gpsimd.dma_start(out=xsl[:, 2:8, :],
                                        in_=xsrc[:, 2:8, :])
                else:
                    nc.sync.dma_start(out=xsl[:],
                                      in_=xsrc[:, kq * 8:(kq + 1) * 8, :])
                for k in range(8):
                    kk = kq * 8 + k
                    st = (kk == 0)
                    sp = (kk == 31)
                    # kv first: their banks free soonest at block turnover
                    for fc in range(2):
                        nc.tensor.matmul(
                            kvps[:, fc, :],
                            Wq_sb[:, kk, (QH + fc) * 128:(QH + fc + 1) * 128],
                            xsl[:, k, :], start=st, stop=sp)
                    for fc in range(QH):
                        nc.tensor.matmul(
                            qps[:, fc, :],
                            Wq_sb[:, kk, fc * 128:(fc + 1) * 128],
                            xsl[:, k, :], start=st, stop=sp)

            # ---- q: evict psum fast (scalar, split so the next block's
            # first accumulations can reuse the early banks sooner)
            qpre = qspool.tile([128, QH, 512], BF16)
            for fc in range(QH):
                nc.scalar.copy(qpre[:, fc, :], qps[:, fc, :])
            for hp in range(2):
                qsl = qpre[:, hp * 2:(hp + 1) * 2, :]
                qshuf = tmppool.tile([128, 2, 512], BF16, name="qshuf",
                                     tag="qshuf", bufs=2)
                nc.vector.stream_shuffle(qshuf[:], qsl, SWAP32)
                t1 = tmppool.tile([128, 2, 512], F32, name="t1", tag="t1",
                                  bufs=2)
                t2 = tmppool.tile([128, 2, 512], F32, name="t2", tag="t2",
                                  bufs=2)
                nc.vector.tensor_mul(t1[:], qsl, rep2(ctabs["cq"][:, sblk, :]))
                nc.vector.tensor_mul(t2[:], qshuf[:],
                                     rep2(ctabs["sq"][:, sblk, :]))
                nc.vector.tensor_add(qT_sb[:, hp * 2:(hp + 1) * 2, tb, :],
                                     t1[:], t2[:])

            # ---- k: RoPE straight from psum
            kshuf = tmppool.tile([128, 512], F32, name="kshuf", tag="kshuf",
                                 bufs=2)
            nc.vector.stream_shuffle(kshuf[:], kvps[:, 0, :], SWAP32)
            u1 = tmppool.tile([128, 512], F32, name="u1", tag="u1", bufs=2)
            u2 = tmppool.tile([128, 512], F32, name="u2", tag="u2", bufs=2)
            nc.vector.tensor_mul(u1[:], kvps[:, 0, :], ctabs["ck"][:, sblk, :])
            nc.vector.tensor_mul(u2[:], kshuf[:], ctabs["sk"][:, sblk, :])
            nc.vector.tensor_add(kT_sb[:, tb, :], u1[:], u2[:])

            # ---- v: evict, transpose back to [tok, feat]
            vT_st = stpool.tile([128, 512], BF16)
            nc.scalar.copy(vT_st[:], kvps[:, 1, :])
            vtp = pstr_pool.tile([128, 512], BF16)
            for i in range(4):
                nc.tensor.transpose(vtp[:, i * 128:(i + 1) * 128],
                                    vT_st[:, i * 128:(i + 1) * 128],
                                    ident_b[:])
            nc.scalar.copy(v_sb[:, tb * 4:(tb + 1) * 4, :], vtp[:])

    # ------- pool for o^T receive, placed over dead Phase-1 space --------
    p3keep = ctx.enter_context(tc.tile_pool(name="p3keep", bufs=1))
    oT_sb = p3keep.tile([128, NH, TSLICE], BF16)   # [feat, hg, tok]
    # f32 rowsums: group 1 on partitions 0-15, group 2 on 32-47 (legal
    # base partitions for the batched reciprocal)
    rraw = p3keep.tile([64, 512], F32)
    rinv = p3keep.tile([64, 512], F32)

    def rbt_tile():
        return p3keep.tile([128, 512], F32, name="rbt", tag="rbt", bufs=4)

    # ---------------- Phase 2: attention per (head, batch, qtg) ----------
    with tc.tile_pool(name="at_z", bufs=1) as zpool, \
         tc.tile_pool(name="at_et", bufs=2) as etpool, \
         tc.tile_pool(name="at_m", bufs=4) as mpool, \
         tc.tile_pool(name="at_o", bufs=6) as opool, \
         tc.tile_pool(name="at_ps_s", bufs=6, space="PSUM") as spsum, \
         tc.tile_pool(name="at_ps_o", bufs=1, space="PSUM") as opsum, \
         tc.tile_pool(name="at_ps_r", bufs=1, space="PSUM") as rpsum:

        if causal:
            # [128,128] upper-triangle additive mask (0 if p<=c else -1e9)
            masktri_sb = zpool.tile([128, 128], F32)
            nc.gpsimd.dma_start(out=masktri_sb[:], in_=mask_e.ap())
        if mask_mode == "general":
            biasB_sb = zpool.tile([128, 1], F32)
            nc.gpsimd.dma_start(out=biasB_sb[:], in_=bias_e.ap())

        def attn_block(h, b, qtg):
            nkt = 4 * (qtg + 1) if causal else 16
            d = b * 4 + qtg
            eT = etpool.tile([128, 16, 512], BF16)
            otps = opsum.tile([128, 512], F32)
            rps = rpsum.tile([128, 512], F32)

            def col0(kt):
                if causal and kt // 4 == qtg:
                    return (kt % 4) * 128
                return 0

            def s_mm(kt):
                c0 = col0(kt)
                tbk = b * 4 + kt // 4
                sps = spsum.tile([128, 512], F32, name="sps", tag="sps")
                nc.tensor.matmul(
                    sps[:, c0:512],
                    kT_sb[:, tbk, (kt % 4) * 128:(kt % 4 + 1) * 128],
                    qT_sb[:, h, b * 4 + qtg, c0:512],
                    start=True, stop=True)
                if causal and kt // 4 == qtg:
                    nc.vector.tensor_add(sps[:, c0:c0 + 128],
                                         sps[:, c0:c0 + 128], masktri_sb[:])
                elif mask_mode == "general":
                    mt = mpool.tile([128, 512], F32)
                    nc.gpsimd.dma_start(
                        out=mt[:],
                        in_=mask_e.ap()[kt * 128:(kt + 1) * 128,
                                        qtg * 512:(qtg + 1) * 512])
                    nc.vector.tensor_add(sps[:], sps[:], mt[:])
                bias = biasB_sb[:] if mask_mode == "general" else 0.0
                nc.scalar.activation(eT[:, kt, c0:512], sps[:, c0:512],
                                     AF.Exp, bias=bias, scale=1.0)

            LA = 6
            for kt in range(min(LA, nkt)):
                s_mm(kt)
            for kt in range(nkt):
                if kt + LA < nkt:
                    s_mm(kt + LA)
                c0 = col0(kt)
                nc.tensor.matmul(otps[:, c0:512], v_sb[:, b * 16 + kt, :],
                                 eT[:, kt, c0:512], start=(kt == 0),
                                 stop=(kt == nkt - 1))
                nc.tensor.matmul(rps[:, c0:512], ones_sb[:],
                                 eT[:, kt, c0:512], start=(kt == 0),
                                 stop=(kt == nkt - 1))

            osb = opool.tile([128, 512], BF16, name="osb", tag="osb")
            nc.vector.tensor_copy(osb[:], otps[:])
            rsb = opool.tile([1, 512], F32, name="rsb", tag="rsb")
            nc.vector.tensor_copy(rsb[:], rps[0:1, :])
            tgt = a2a_in1 if h < 2 else a2a_in2
            hl = h % 2
            nc.sync.dma_start(out=tgt[d, hl * 128:(hl + 1) * 128, :],
                              in_=osb[:])
            nc.sync.dma_start(out=tgt[d, 256 + hl * 2:258 + hl * 2, :],
                              in_=rsb[:].bitcast(BF16))

        for h in range(QH):
            for b in range(B):
                for qtg in range(4):
                    attn_block(h, b, qtg)
            if h == 1:
                # gpsimd queue is otherwise empty in P2, so the trigger
                # waiting at its head costs nothing; completes during
                # head-2/3 compute and the receive below hides too
                nc.gpsimd.collective_compute(
                    "AllToAll", OP.bypass,
                    replica_groups=[list(range(NCORES))],
                    ins=[a2a_in1.opt()], outs=[a2a_out1.opt()])

        # group-1 o^T + rowsums land while heads 2/3 still compute
        for s in range(NCORES):
            for hl in range(2):
                nc.gpsimd.dma_start(
                    out=oT_sb[:, s * QH + hl, :],
                    in_=a2a_out1[s, hl * 128:(hl + 1) * 128, :])
                nc.gpsimd.dma_start(
                    out=rraw[s * 2 + hl:s * 2 + hl + 1, :],
                    in_=a2a_out1[s, 256 + hl * 2:258 + hl * 2, :]
                    .bitcast(F32))
        nc.vector.reciprocal(rinv[0:16, :], rraw[0:16, :])
        nc.sync.dma_start(out=rdram[0], in_=rinv[0:16, :])
        # broadcast 1/r over partitions via stride-0 DRAM reads, then
        # normalize group-1 o^T in place
        for s in range(NCORES):
            for hl in range(2):
                hg = s * QH + hl
                rbt = rbt_tile()
                nc.gpsimd.dma_start(
                    out=rbt[:],
                    in_=bass.AP(tensor=rdram.tensor,
                                offset=rdram.offset + (s * 2 + hl) * 512,
                                ap=[[0, 128], [1, 512]]))
                # gpsimd: the vector queue is saturated with P2 mask adds
                nc.gpsimd.tensor_mul(oT_sb[:, hg, :], oT_sb[:, hg, :],
                                     rbt[:])

        nc.gpsimd.collective_compute(
            "AllToAll", OP.bypass,
            replica_groups=[list(range(NCORES))],
            ins=[a2a_in2.opt()], outs=[a2a_out2.opt()])

    # ---------------- Phase 3: Wo GEMM on own token slice ----------------
    with tc.tile_pool(name="wo_acc", bufs=1) as accpool, \
         tc.tile_pool(name="wo_w", bufs=20) as wopool, \
         tc.tile_pool(name="wo_out", bufs=6) as outpool, \
         tc.tile_pool(name="wo_ps", bufs=2, space="PSUM") as wopsum:

        for s in range(NCORES):
            for hl in range(2):
                nc.gpsimd.dma_start(
                    out=oT_sb[:, s * QH + 2 + hl, :],
                    in_=a2a_out2[s, hl * 128:(hl + 1) * 128, :])
                nc.gpsimd.dma_start(
                    out=rraw[32 + s * 2 + hl:32 + s * 2 + hl + 1, :],
                    in_=a2a_out2[s, 256 + hl * 2:258 + hl * 2, :]
                    .bitcast(F32))
        nc.vector.reciprocal(rinv[32:48, :], rraw[32:48, :])
        nc.sync.dma_start(out=rdram[1], in_=rinv[32:48, :])
        for s in range(NCORES):
            for hl in range(2):
                hg = s * QH + 2 + hl
                rbt = rbt_tile()
                dq = (nc.gpsimd, nc.scalar, nc.sync)[(s * 2 + hl) % 3]
                dq.dma_start(
                    out=rbt[:],
                    in_=bass.AP(tensor=rdram.tensor,
                                offset=rdram.offset + (16 + s * 2 + hl) * 512,
                                ap=[[0, 128], [1, 512]]))
                nc.vector.tensor_mul(oT_sb[:, hg, :], oT_sb[:, hg, :],
                                     rbt[:])

        grp1 = [s * QH + g for s in range(NCORES) for g in (0, 1)]
        grp2 = [s * QH + 2 + g for s in range(NCORES) for g in (0, 1)]
        accs = []
        # pass 1: group-1 contraction only -> SBUF accumulators, giving
        # the second AllToAll a full half-GEMM to hide under
        for dc in range(8):
            pso = wopsum.tile([128, 4, 512], F32, name="pso", tag="pso")
            for ci, hg in enumerate(grp1):
                wot = wopool.tile([128, 512], BF16)
                dq = nc.scalar if ci % 2 == 0 else nc.sync
                dq.dma_start(out=wot[:],
                             in_=Wo_e.ap()[hg * 128:(hg + 1) * 128,
                                           dc * 512:(dc + 1) * 512])
                for t in range(4):
                    nc.tensor.matmul(pso[:, t, :],
                                     oT_sb[:, hg, t * 128:(t + 1) * 128],
                                     wot[:], start=(ci == 0), stop=(ci == 15))
            acc = accpool.tile([128, 4, 512], F32, name=f"acc{dc}",
                               tag=f"acc{dc}")
            accs.append(acc)
            nc.vector.tensor_copy(acc[:], pso[:])
        # pass 2: group-2 contraction + partial sums -> output
        for dc in range(8):
            pso = wopsum.tile([128, 4, 512], F32, name="pso", tag="pso")
            for ci, hg in enumerate(grp2):
                wot = wopool.tile([128, 512], BF16)
                dq = (nc.gpsimd, nc.scalar, nc.sync)[ci % 3]
                dq.dma_start(out=wot[:],
                             in_=Wo_e.ap()[hg * 128:(hg + 1) * 128,
                                           dc * 512:(dc + 1) * 512])
                for t in range(4):
                    nc.tensor.matmul(pso[:, t, :],
                                     oT_sb[:, hg, t * 128:(t + 1) * 128],
                                     wot[:], start=(ci == 0), stop=(ci == 15))
            for t in range(4):
                osb = outpool.tile([128, 512], F32)
                nc.vector.tensor_add(osb[:], accs[dc][:, t, :], pso[:, t, :])
                nc.sync.dma_start(
                    out=out_e.ap()[t * 128:(t + 1) * 128,
                                   dc * 512:(dc + 1) * 512],
                    in_=osb[:])


_NC_CACHE = {}


def _get_nc(mask_mode):
    if mask_mode not in _NC_CACHE:
        _NC_CACHE[mask_mode] = _build(mask_mode)
    return _NC_CACHE[mask_mode]


def _estimate_score_bound(x, Wq, Wk, fc, fs):
    """Sampled upper estimate of max |q.k/sqrt(hd)| after RoPE."""
    rng = np.random.default_rng(12345)
    x2 = x.reshape(TOK, D)
    rq = rng.choice(TOK, 192, replace=False)
    rk = rng.choice(TOK, 192, replace=False)
    q = (x2[rq] @ Wq).reshape(192, NH, HD)
    k = (x2[rk] @ Wk).reshape(192, NKV, HD)

    def rope(t, pos):
        c, s = fc[pos % S], fs[pos % S]
        tr, ti = t[..., 0::2], t[..., 1::2]
        out = np.empty_like(t)
        out[..., 0::2] = tr * c[:, None, :] - ti * s[:, None, :]
        out[..., 1::2] = tr * s[:, None, :] + ti * c[:, None, :]
        return out

    q = rope(q, rq)
    k = rope(k, rk)
    qg = q.reshape(192, NKV, NH // NKV, HD)
    sc = np.einsum('qgnd,kgd->gnqk', qg, k) / np.float32(math.sqrt(HD))
    return float(np.abs(sc).max())


def _rope_tables(freqs_cos, freqs_sin, scale):
    """[128, S] transposed tables: c duplicated on partition pairs,
    s with -sin on even / +sin on odd partitions."""
    c = np.empty((128, S), np.float32)
    s = np.empty((128, S), np.float32)
    c[0::2] = c[1::2] = freqs_cos.T * scale
    s[0::2] = -freqs_sin.T * scale
    s[1::2] = freqs_sin.T * scale
    return np.ascontiguousarray(c), np.ascontiguousarray(s)


def kernel(x, Wq, Wk, Wv, Wo, freqs_cos, freqs_sin, mask, start_pos=0,
           _want_trace=False):
    x = np.asarray(x, dtype=np.float32)
    mask = np.asarray(mask, dtype=np.float32)
    freqs_cos = np.asarray(freqs_cos, dtype=np.float32)
    freqs_sin = np.asarray(freqs_sin, dtype=np.float32)

    if not mask.any():
        mask_mode = "zeros"
    else:
        canon = np.where(np.tril(np.ones((S, S), bool)), 0.0,
                         np.float32(NEG_INF)).astype(np.float32)
        mask_mode = "causal" if np.array_equal(mask, canon) else "general"

    xT = np.ascontiguousarray(x.reshape(TOK, D).T.astype(ml_dtypes.bfloat16))
    scale = np.float32(1.0 / math.sqrt(HD))
    cqT, sqT = _rope_tables(freqs_cos, freqs_sin, scale)
    ckT, skT = _rope_tables(freqs_cos, freqs_sin, np.float32(1.0))
    Wo_bf = np.ascontiguousarray(np.asarray(Wo, np.float32)
                                 .astype(ml_dtypes.bfloat16))
    if mask_mode == "causal":
        # [128,128] upper-triangle additive mask in transposed layout
        p = np.arange(128)[:, None]
        q = np.arange(128)[None, :]
        masktri = np.where(p <= q, 0.0, NEG_INF).astype(np.float32)
        masktri = np.ascontiguousarray(masktri)

    in_maps = []
    for c in range(NCORES):
        m = {
            "xT": xT,
            "Wqkv": np.ascontiguousarray(np.concatenate(
                [Wq[:, c * QH * HD:(c + 1) * QH * HD],
                 Wk[:, c * HD:(c + 1) * HD],
                 Wv[:, c * HD:(c + 1) * HD]],
                axis=1).astype(ml_dtypes.bfloat16)),
            "Wo": Wo_bf,
            "cqT": cqT, "sqT": sqT, "ckT": ckT, "skT": skT,
        }
        if mask_mode == "causal":
            m["masktri"] = masktri
        elif mask_mode == "general":
            m["maskT"] = np.ascontiguousarray(mask.T)
            bound = _estimate_score_bound(x, Wq, Wk, freqs_cos, freqs_sin)
            bmax = float(np.max(mask[np.isfinite(mask)])) if \
                np.isfinite(mask).any() else 0.0
            m["biasB"] = np.full((128, 1), -(1.25 * bound + max(bmax, 0.0)),
                                 np.float32)
        in_maps.append(m)

    nc = _get_nc(mask_mode)
    res = run_bass_kernel_spmd(nc, in_maps, list(range(NCORES)),
                               trace=_want_trace)
    out = np.concatenate([res.results[c]["out"] for c in range(NCORES)],
                         axis=0).reshape(B, S, D)
    if _want_trace:
        return out, res
    return out


if __name__ == "__main__":
    rng = np.random.default_rng(0)
    x = rng.standard_normal((B, S, D), dtype=np.float32) * 0.1
    Wq = rng.standard_normal((D, NH * HD), dtype=np.float32) * 0.02
    Wk = rng.standard_normal((D, NKV * HD), dtype=np.float32) * 0.02
    Wv = rng.standard_normal((D, NKV * HD), dtype=np.float32) * 0.02
    Wo = rng.standard_normal((NH * HD, D), dtype=np.float32) * 0.02
    fc = rng.standard_normal((S, 64), dtype=np.float32)
    fs = rng.standard_normal((S, 64), dtype=np.float32)
    mask = np.where(np.tril(np.ones((S, S), bool)), 0.0,
                    np.float32(NEG_INF)).astype(np.float32)
    out = kernel(x, Wq, Wk, Wv, Wo, fc, fs, mask, 0)
    print("out", out.shape, out.dtype, np.abs(out).mean())


# revision 26
# speedup vs baseline: 1.0042x; 1.0042x over previous
"""Distributed Trainium2 kernel for GQA attention (B=2, S=2048, D=4096,
32 q-heads / 8 kv-heads, HD=128, RoPE, additive causal mask) on 8
NeuronCores.

Sharding: tensor-parallel over heads (4 q-heads + 1 kv-head per core).

Phase 1 computes the QKV projections weights-stationary so q^T/k^T come
out directly in [feat, tok] layout (RoPE applied in that layout via a
partition pair-swap stream shuffle); v is rotated back to [tok, feat]
with PE transposes. Phase 2 computes scores transposed (k stationary,
q^T moving -> s^T tiles), uses a max-free softmax (exp straight out of
PSUM, causal mask added only on diagonal tiles), feeds e^T directly to
the pv matmul (no p transposes), and accumulates per-row softmax
denominators with a ones-stationary matmul. Unnormalized o^T plus the
f32 row sums ride two AllToAlls (after head pair 0/1 and 2/3). Phase 3
normalizes the received o^T by the broadcast reciprocal row sums and
runs the Wo GEMM for this core's 512-token slice, ordering the
contraction so the second AllToAll hides under the first half's
compute. All matmuls bf16 with f32 PSUM accumulation.
"""
import sys

sys.path.insert(0, "/opt/trn_rl_repo")

import math
from contextlib import ExitStack
import numpy as np
import ml_dtypes

import concourse.bass as bass
import concourse.tile as tile
from concourse import bacc, mybir
from concourse.bass_utils import run_bass_kernel_spmd
from concourse.masks import make_identity

F32 = mybir.dt.float32
BF16 = mybir.dt.bfloat16
AF = mybir.ActivationFunctionType
OP = mybir.AluOpType

NCORES = 8
B, S, D = 2, 2048, 4096
NH, NKV, HD = 32, 8, 128
QH = NH // NCORES          # 4 q-heads per core
TOK = B * S                # 4096
TB = TOK // 512            # 8 blocks of 512 tokens
TT = TOK // 128            # 32 token tiles
TSLICE = TOK // NCORES     # 512 tokens out per core
NEG_INF = -1e9
SWAP32 = [i ^ 1 for i in range(32)]   # even/odd partition pair swap


def _build(mask_mode: str):
    nc = bacc.Bacc("TRN2", target_bir_lowering=False, debug=False,
                   enable_asserts=False, num_devices=NCORES)

    xT_e = nc.dram_tensor("xT", [D, TOK], BF16, kind="ExternalInput")
    Wq_e = nc.dram_tensor("Wqkv", [D, (QH + 2) * HD], BF16,
                          kind="ExternalInput")
    Wo_e = nc.dram_tensor("Wo", [D, D], BF16, kind="ExternalInput")
    # RoPE tables in transposed layout [feat 128, pos]: cq/sq q-scaled,
    # sq/sk carry the parity sign (-sin on even, +sin on odd partitions).
    cq_e = nc.dram_tensor("cqT", [128, S], F32, kind="ExternalInput")
    sq_e = nc.dram_tensor("sqT", [128, S], F32, kind="ExternalInput")
    ck_e = nc.dram_tensor("ckT", [128, S], F32, kind="ExternalInput")
    sk_e = nc.dram_tensor("skT", [128, S], F32, kind="ExternalInput")
    if mask_mode == "causal":
        mask_e = nc.dram_tensor("masktri", [128, 128], F32,
                                kind="ExternalInput")
    elif mask_mode == "general":
        mask_e = nc.dram_tensor("maskT", [S, S], F32, kind="ExternalInput")
    else:
        mask_e = None
    bias_e = nc.dram_tensor("biasB", [128, 1], F32, kind="ExternalInput") \
        if mask_mode == "general" else None
    out_e = nc.dram_tensor("out", [TSLICE, D], F32, kind="ExternalOutput")

    with tile.TileContext(nc) as tc, ExitStack() as ctx:
        _body(ctx, tc, mask_mode, xT_e, Wq_e, Wo_e,
              cq_e, sq_e, ck_e, sk_e, mask_e, bias_e, out_e)
    nc.compile()
    return nc


def _body(ctx, tc, mask_mode, xT_e, Wq_e, Wo_e,
          cq_e, sq_e, ck_e, sk_e, mask_e, bias_e, out_e):
    nc = tc.nc
    causal = mask_mode == "causal"

    consts = ctx.enter_context(tc.tile_pool(name="consts", bufs=1))
    ident_b = consts.tile([128, 128], BF16)
    make_identity(nc, ident_b[:])
    ones_sb = consts.tile([128, 128], BF16)
    nc.vector.memset(ones_sb[:], 1.0)

    # persistent across phases
    kv_pool = ctx.enter_context(tc.tile_pool(name="kv", bufs=1))
    qT_sb = kv_pool.tile([128, QH, TB, 512], BF16)  # [hd, h, tb, tok]
    kT_sb = kv_pool.tile([128, TB, 512], BF16)      # [hd, tb, tok]
    v_sb = kv_pool.tile([128, TT, 128], BF16)       # [tok, kt, feat]

    dram = ctx.enter_context(tc.tile_pool(name="dram", bufs=1, space="DRAM"))
    # per slot: 256 rows of o^T (2 heads) + 4 rows = 2x[1,512] f32 rowsums
    a2a_in1 = dram.tile([NCORES, 260, TSLICE], BF16)
    a2a_out1 = dram.tile([NCORES, 260, TSLICE], BF16)
    a2a_in2 = dram.tile([NCORES, 260, TSLICE], BF16)
    a2a_out2 = dram.tile([NCORES, 260, TSLICE], BF16)
    rdram = dram.tile([2, 16, 512], F32)

    # ---------------- Phase 1: QKV projections + RoPE --------------------
    with tc.tile_pool(name="pj_w", bufs=1) as wpool, \
         tc.tile_pool(name="pj_x", bufs=2) as xpool, \
         tc.tile_pool(name="pj_rt", bufs=1) as rtpool, \
         tc.tile_pool(name="pj_qs", bufs=2) as qspool, \
         tc.tile_pool(name="pj_tmp", bufs=1) as tmppool, \
         tc.tile_pool(name="pj_st", bufs=3) as stpool, \
         tc.tile_pool(name="pj_psq", bufs=1, space="PSUM") as psq_pool, \
         tc.tile_pool(name="pj_pskv", bufs=1, space="PSUM") as pskv_pool, \
         tc.tile_pool(name="pj_pstr", bufs=2, space="PSUM") as pstr_pool:

        Wq_sb = wpool.tile([128, 32, (QH + 2) * HD], BF16)
        wq_re = Wq_e.ap().rearrange("(k p) f -> p k f", p=128)
        # split the weight load so the first k-slices land fast
        nc.sync.dma_start(out=Wq_sb[:, 0:1, :], in_=wq_re[:, 0:1, :])
        nc.sync.dma_start(out=Wq_sb[:, 1:4, :], in_=wq_re[:, 1:4, :])
        nc.sync.dma_start(out=Wq_sb[:, 4:16, :], in_=wq_re[:, 4:16, :])
        nc.sync.dma_start(out=Wq_sb[:, 16:32, :], in_=wq_re[:, 16:32, :])

        ctabs = {}
        for nm, te in (("cq", cq_e), ("sq", sq_e), ("ck", ck_e), ("sk", sk_e)):
            t = rtpool.tile([128, 4, 512], F32, name=f"tab_{nm}",
                            tag=f"tab_{nm}")
            nc.scalar.dma_start(
                out=t[:], in_=te.ap().rearrange("p (sb t) -> p sb t", t=512))
            ctabs[nm] = t

        def rep2(ap):  # repeat a [128, 512] table slice 2x along free dim
            return bass.AP(tensor=ap.tensor, offset=ap.offset,
                           ap=[ap.ap[0], [0, 2], ap.ap[-1]])

        for tb in range(TB):
            sblk = tb % 4   # position block within batch
            qps = psq_pool.tile([128, QH, 512], F32)
            kvps = pskv_pool.tile([128, 2, 512], F32)
            for kq in range(4):
                xsl = xpool.tile([128, 8, 512], BF16)
                xsrc = xT_e.ap()[:, tb * 512:(tb + 1) * 512] \
                    .rearrange("(k p) t -> p k t", p=128)
                if tb == 0 and kq == 0:
                    # split the very first load so matmuls start sooner
                    nc.gpsimd.dma_start(out=xsl[:, 0:2, :],
                                        in_=xsrc[:, 0:2, :])
                    nc.gpsimd.dma_start(out=xsl[:, 2:8, :],
                                        in_=xsrc[:, 2:8, :])
                else:
                    nc.sync.dma_start(out=xsl[:],
                                      in_=xsrc[:, kq * 8:(kq + 1) * 8, :])
                for k in range(8):
                    kk = kq * 8 + k
                    st = (kk == 0)
                    sp = (kk == 31)
                    # kv first: their banks free soonest at block turnover
                    for fc in range(2):
                        nc.tensor.matmul(
                            kvps[:, fc, :],
                            Wq_sb[:, kk, (QH + fc) * 128:(QH + fc + 1) * 128],
                            xsl[:, k, :], start=st, stop=sp)
                    for fc in range(QH):
                        nc.tensor.matmul(
                            qps[:, fc, :],
                            Wq_sb[:, kk, fc * 128:(fc + 1) * 128],
                            xsl[:, k, :], start=st, stop=sp)

            # ---- q: evict psum fast (scalar, split so the next block's
            # first accumulations can reuse the early banks sooner)
            qpre = qspool.tile([128, QH, 512], BF16)
            for fc in range(QH):
                nc.scalar.copy(qpre[:, fc, :], qps[:, fc, :])
            for hp in range(2):
                qsl = qpre[:, hp * 2:(hp + 1) * 2, :]
                qshuf = tmppool.tile([128, 2, 512], BF16, name="qshuf",
                                     tag="qshuf", bufs=2)
                nc.vector.stream_shuffle(qshuf[:], qsl, SWAP32)
                t1 = tmppool.tile([128, 2, 512], F32, name="t1", tag="t1",
                                  bufs=2)
                t2 = tmppool.tile([128, 2, 512], F32, name="t2", tag="t2",
                                  bufs=2)
                nc.vector.tensor_mul(t1[:], qsl, rep2(ctabs["cq"][:, sblk, :]))
                nc.vector.tensor_mul(t2[:], qshuf[:],
                                     rep2(ctabs["sq"][:, sblk, :]))
                nc.vector.tensor_add(qT_sb[:, hp * 2:(hp + 1) * 2, tb, :],
                                     t1[:], t2[:])

            # ---- k: RoPE straight from psum
            kshuf = tmppool.tile([128, 512], F32, name="kshuf", tag="kshuf",
                                 bufs=2)
            nc.vector.stream_shuffle(kshuf[:], kvps[:, 0, :], SWAP32)
            u1 = tmppool.tile([128, 512], F32, name="u1", tag="u1", bufs=2)
            u2 = tmppool.tile([128, 512], F32, name="u2", tag="u2", bufs=2)
            nc.vector.tensor_mul(u1[:], kvps[:, 0, :], ctabs["ck"][:, sblk, :])
            nc.vector.tensor_mul(u2[:], kshuf[:], ctabs["sk"][:, sblk, :])
            nc.vector.tensor_add(kT_sb[:, tb, :], u1[:], u2[:])

            # ---- v: evict, transpose back to [tok, feat]
            vT_st = stpool.tile([128, 512], BF16)
            nc.scalar.copy(vT_st[:], kvps[:, 1, :])
            vtp = pstr_pool.tile([128, 512], BF16)
            for i in range(4):
                nc.tensor.transpose(vtp[:, i * 128:(i + 1) * 128],
                                    vT_st[:, i * 128:(i + 1) * 128],
                                    ident_b[:])
            nc.scalar.copy(v_sb[:, tb * 4:(tb + 1) * 4, :], vtp[:])

    # ------- pool for o^T receive, placed over dead Phase-1 space --------
    p3keep = ctx.enter_context(tc.tile_pool(name="p3keep", bufs=1))
    oT_sb = p3keep.tile([128, NH, TSLICE], BF16)   # [feat, hg, tok]
    # f32 rowsums: group 1 on partitions 0-15, group 2 on 32-47 (legal
    # base partitions for the batched reciprocal)
    rraw = p3keep.tile([64, 512], F32)
    rinv = p3keep.tile([64, 512], F32)

    def rbt_tile():
        return p3keep.tile([128, 512], F32, name="rbt", tag="rbt", bufs=4)

    # ---------------- Phase 2: attention per (head, batch, qtg) ----------
    with tc.tile_pool(name="at_z", bufs=1) as zpool, \
         tc.tile_pool(name="at_et", bufs=2) as etpool, \
         tc.tile_pool(name="at_m", bufs=4) as mpool, \
         tc.tile_pool(name="at_o", bufs=6) as opool, \
         tc.tile_pool(name="at_ps_s", bufs=5, space="PSUM") as spsum, \
         tc.tile_pool(name="at_ps_o", bufs=2, space="PSUM") as opsum, \
         tc.tile_pool(name="at_ps_r", bufs=1, space="PSUM") as rpsum:

        if causal:
            # [128,128] upper-triangle additive mask (0 if p<=c else -1e9)
            masktri_sb = zpool.tile([128, 128], F32)
            nc.gpsimd.dma_start(out=masktri_sb[:], in_=mask_e.ap())
        if mask_mode == "general":
            biasB_sb = zpool.tile([128, 1], F32)
            nc.gpsimd.dma_start(out=biasB_sb[:], in_=bias_e.ap())

        def attn_block(h, b, qtg):
            nkt = 4 * (qtg + 1) if causal else 16
            d = b * 4 + qtg
            eT = etpool.tile([128, 16, 512], BF16)
            otps = opsum.tile([128, 512], F32)
            rps = rpsum.tile([128, 512], F32)

            def col0(kt):
                if causal and kt // 4 == qtg:
                    return (kt % 4) * 128
                return 0

            def s_mm(kt):
                c0 = col0(kt)
                tbk = b * 4 + kt // 4
                sps = spsum.tile([128, 512], F32, name="sps", tag="sps")
                nc.tensor.matmul(
                    sps[:, c0:512],
                    kT_sb[:, tbk, (kt % 4) * 128:(kt % 4 + 1) * 128],
                    qT_sb[:, h, b * 4 + qtg, c0:512],
                    start=True, stop=True)
                if causal and kt // 4 == qtg:
                    nc.vector.tensor_add(sps[:, c0:c0 + 128],
                                         sps[:, c0:c0 + 128], masktri_sb[:])
                elif mask_mode == "general":
                    mt = mpool.tile([128, 512], F32)
                    nc.gpsimd.dma_start(
                        out=mt[:],
                        in_=mask_e.ap()[kt * 128:(kt + 1) * 128,
                                        qtg * 512:(qtg + 1) * 512])
                    nc.vector.tensor_add(sps[:], sps[:], mt[:])
                bias = biasB_sb[:] if mask_mode == "general" else 0.0
                nc.scalar.activation(eT[:, kt, c0:512], sps[:, c0:512],
                                     AF.Exp, bias=bias, scale=1.0)

            LA = 5
            for kt in range(min(LA, nkt)):
                s_mm(kt)
            for kt in range(nkt):
                if kt + LA < nkt:
                    s_mm(kt + LA)
                c0 = col0(kt)
                nc.tensor.matmul(otps[:, c0:512], v_sb[:, b * 16 + kt, :],
                                 eT[:, kt, c0:512], start=(kt == 0),
                                 stop=(kt == nkt - 1))
                nc.tensor.matmul(rps[:, c0:512], ones_sb[:],
                                 eT[:, kt, c0:512], start=(kt == 0),
                                 stop=(kt == nkt - 1))

            osb = opool.tile([128, 512], BF16, name="osb", tag="osb")
            nc.vector.tensor_copy(osb[:], otps[:])
            rsb = opool.tile([1, 512], F32, name="rsb", tag="rsb")
            nc.vector.tensor_copy(rsb[:], rps[0:1, :])
            tgt = a2a_in1 if h < 2 else a2a_in2
            hl = h % 2
            nc.sync.dma_start(out=tgt[d, hl * 128:(hl + 1) * 128, :],
                              in_=osb[:])
            nc.sync.dma_start(out=tgt[d, 256 + hl * 2:258 + hl * 2, :],
                              in_=rsb[:].bitcast(BF16))

        for h in range(QH):
            for b in range(B):
                for qtg in range(4):
                    attn_block(h, b, qtg)
            if h == 1:
                # gpsimd queue is otherwise empty in P2, so the trigger
                # waiting at its head costs nothing; completes during
                # head-2/3 compute and the receive below hides too
                nc.gpsimd.collective_compute(
                    "AllToAll", OP.bypass,
                    replica_groups=[list(range(NCORES))],
                    ins=[a2a_in1.opt()], outs=[a2a_out1.opt()])

        # group-1 o^T + rowsums land while heads 2/3 still compute
        for s in range(NCORES):
            for hl in range(2):
                nc.gpsimd.dma_start(
                    out=oT_sb[:, s * QH + hl, :],
                    in_=a2a_out1[s, hl * 128:(hl + 1) * 128, :])
                nc.gpsimd.dma_start(
                    out=rraw[s * 2 + hl:s * 2 + hl + 1, :],
                    in_=a2a_out1[s, 256 + hl * 2:258 + hl * 2, :]
                    .bitcast(F32))
        nc.vector.reciprocal(rinv[0:16, :], rraw[0:16, :])
        nc.sync.dma_start(out=rdram[0], in_=rinv[0:16, :])
        # broadcast 1/r over partitions via stride-0 DRAM reads, then
        # normalize group-1 o^T in place
        for s in range(NCORES):
            for hl in range(2):
                hg = s * QH + hl
                rbt = rbt_tile()
                nc.gpsimd.dma_start(
                    out=rbt[:],
                    in_=bass.AP(tensor=rdram.tensor,
                                offset=rdram.offset + (s * 2 + hl) * 512,
                                ap=[[0, 128], [1, 512]]))
                # gpsimd: the vector queue is saturated with P2 mask adds
                nc.gpsimd.tensor_mul(oT_sb[:, hg, :], oT_sb[:, hg, :],
                                     rbt[:])

        nc.gpsimd.collective_compute(
            "AllToAll", OP.bypass,
            replica_groups=[list(range(NCORES))],
            ins=[a2a_in2.opt()], outs=[a2a_out2.opt()])

    # ---------------- Phase 3: Wo GEMM on own token slice ----------------
    with tc.tile_pool(name="wo_acc", bufs=1) as accpool, \
         tc.tile_pool(name="wo_w", bufs=20) as wopool, \
         tc.tile_pool(name="wo_out", bufs=6) as outpool, \
         tc.tile_pool(name="wo_ps", bufs=2, space="PSUM") as wopsum:

        for s in range(NCORES):
            for hl in range(2):
                nc.gpsimd.dma_start(
                    out=oT_sb[:, s * QH + 2 + hl, :],
                    in_=a2a_out2[s, hl * 128:(hl + 1) * 128, :])
                nc.gpsimd.dma_start(
                    out=rraw[32 + s * 2 + hl:32 + s * 2 + hl + 1, :],
                    in_=a2a_out2[s, 256 + hl * 2:258 + hl * 2, :]
                    .bitcast(F32))
        nc.vector.reciprocal(rinv[32:48, :], rraw[32:48, :])
        nc.sync.dma_start(out=rdram[1], in_=rinv[32:48, :])
        for s in range(NCORES):
            for hl in range(2):
                hg = s * QH + 2 + hl
                rbt = rbt_tile()
                dq = (nc.gpsimd, nc.scalar, nc.sync)[(s * 2 + hl) % 3]
                dq.dma_start(
                    out=rbt[:],
                    in_=bass.AP(tensor=rdram.tensor,
                                offset=rdram.offset + (16 + s * 2 + hl) * 512,
                                ap=[[0, 128], [1, 512]]))
                nc.vector.tensor_mul(oT_sb[:, hg, :], oT_sb[:, hg, :],
                                     rbt[:])

        grp1 = [s * QH + g for s in range(NCORES) for g in (0, 1)]
        grp2 = [s * QH + 2 + g for s in range(NCORES) for g in (0, 1)]
        accs = []
        # pass 1: group-1 contraction only -> SBUF accumulators, giving
        # the second AllToAll a full half-GEMM to hide under
        for dc in range(8):
            pso = wopsum.tile([128, 4, 512], F32, name="pso", tag="pso")
            for ci, hg in enumerate(grp1):
                wot = wopool.tile([128, 512], BF16)
                dq = nc.scalar if ci % 2 == 0 else nc.sync
                dq.dma_start(out=wot[:],
                             in_=Wo_e.ap()[hg * 128:(hg + 1) * 128,
                                           dc * 512:(dc + 1) * 512])
                for t in range(4):
                    nc.tensor.matmul(pso[:, t, :],
                                     oT_sb[:, hg, t * 128:(t + 1) * 128],
                                     wot[:], start=(ci == 0), stop=(ci == 15))
            acc = accpool.tile([128, 4, 512], F32, name=f"acc{dc}",
                               tag=f"acc{dc}")
            accs.append(acc)
            nc.vector.tensor_copy(acc[:], pso[:])
        # pass 2: group-2 contraction + partial sums -> output
        for dc in range(8):
            pso = wopsum.tile([128, 4, 512], F32, name="pso", tag="pso")
            for ci, hg in enumerate(grp2):
                wot = wopool.tile([128, 512], BF16)
                dq = (nc.gpsimd, nc.scalar, nc.sync)[ci % 3]
                dq.dma_start(out=wot[:],
                             in_=Wo_e.ap()[hg * 128:(hg + 1) * 128,
                                           dc * 512:(dc + 1) * 512])
                for t in range(4):
                    nc.tensor.matmul(pso[:, t, :],
                                     oT_sb[:, hg, t * 128:(t + 1) * 128],
                                     wot[:], start=(ci == 0), stop=(ci == 15))
            for t in range(4):
                osb = outpool.tile([128, 512], F32)
                nc.vector.tensor_add(osb[:], accs[dc][:, t, :], pso[:, t, :])
                nc.sync.dma_start(
                    out=out_e.ap()[t * 128:(t + 1) * 128,
                                   dc * 512:(dc + 1) * 512],
                    in_=osb[:])


_NC_CACHE = {}


def _get_nc(mask_mode):
    if mask_mode not in _NC_CACHE:
        _NC_CACHE[mask_mode] = _build(mask_mode)
    return _NC_CACHE[mask_mode]


def _estimate_score_bound(x, Wq, Wk, fc, fs):
    """Sampled upper estimate of max |q.k/sqrt(hd)| after RoPE."""
    rng = np.random.default_rng(12345)
    x2 = x.reshape(TOK, D)
    rq = rng.choice(TOK, 192, replace=False)
    rk = rng.choice(TOK, 192, replace=False)
    q = (x2[rq] @ Wq).reshape(192, NH, HD)
    k = (x2[rk] @ Wk).reshape(192, NKV, HD)

    def rope(t, pos):
        c, s = fc[pos % S], fs[pos % S]
        tr, ti = t[..., 0::2], t[..., 1::2]
        out = np.empty_like(t)
        out[..., 0::2] = tr * c[:, None, :] - ti * s[:, None, :]
        out[..., 1::2] = tr * s[:, None, :] + ti * c[:, None, :]
        return out

    q = rope(q, rq)
    k = rope(k, rk)
    qg = q.reshape(192, NKV, NH // NKV, HD)
    sc = np.einsum('qgnd,kgd->gnqk', qg, k) / np.float32(math.sqrt(HD))
    return float(np.abs(sc).max())


def _rope_tables(freqs_cos, freqs_sin, scale):
    """[128, S] transposed tables: c duplicated on partition pairs,
    s with -sin on even / +sin on odd partitions."""
    c = np.empty((128, S), np.float32)
    s = np.empty((128, S), np.float32)
    c[0::2] = c[1::2] = freqs_cos.T * scale
    s[0::2] = -freqs_sin.T * scale
    s[1::2] = freqs_sin.T * scale
    return np.ascontiguousarray(c), np.ascontiguousarray(s)


def kernel(x, Wq, Wk, Wv, Wo, freqs_cos, freqs_sin, mask, start_pos=0,
           _want_trace=False):
    x = np.asarray(x, dtype=np.float32)
    mask = np.asarray(mask, dtype=np.float32)
    freqs_cos = np.asarray(freqs_cos, dtype=np.float32)
    freqs_sin = np.asarray(freqs_sin, dtype=np.float32)

    if not mask.any():
        mask_mode = "zeros"
    else:
        canon = np.where(np.tril(np.ones((S, S), bool)), 0.0,
                         np.float32(NEG_INF)).astype(np.float32)
        mask_mode = "causal" if np.array_equal(mask, canon) else "general"

    xT = np.ascontiguousarray(x.reshape(TOK, D).T.astype(ml_dtypes.bfloat16))
    scale = np.float32(1.0 / math.sqrt(HD))
    cqT, sqT = _rope_tables(freqs_cos, freqs_sin, scale)
    ckT, skT = _rope_tables(freqs_cos, freqs_sin, np.float32(1.0))
    Wo_bf = np.ascontiguousarray(np.asarray(Wo, np.float32)
                                 .astype(ml_dtypes.bfloat16))
    if mask_mode == "causal":
        # [128,128] upper-triangle additive mask in transposed layout
        p = np.arange(128)[:, None]
        q = np.arange(128)[None, :]
        masktri = np.where(p <= q, 0.0, NEG_INF).astype(np.float32)
        masktri = np.ascontiguousarray(masktri)

    in_maps = []
    for c in range(NCORES):
        m = {
            "xT": xT,
            "Wqkv": np.ascontiguousarray(np.concatenate(
                [Wq[:, c * QH * HD:(c + 1) * QH * HD],
                 Wk[:, c * HD:(c + 1) * HD],
                 Wv[:, c * HD:(c + 1) * HD]],
                axis=1).astype(ml_dtypes.bfloat16)),
            "Wo": Wo_bf,
            "cqT": cqT, "sqT": sqT, "ckT": ckT, "skT": skT,
        }
        if mask_mode == "causal":
            m["masktri"] = masktri
        elif mask_mode == "general":
            m["maskT"] = np.ascontiguousarray(mask.T)
            bound = _estimate_score_bound(x, Wq, Wk, freqs_cos, freqs_sin)
            bmax = float(np.max(mask[np.isfinite(mask)])) if \
                np.isfinite(mask).any() else 0.0
            m["biasB"] = np.full((128, 1), -(1.25 * bound + max(bmax, 0.0)),
                                 np.float32)
        in_maps.append(m)

    nc = _get_nc(mask_mode)
    res = run_bass_kernel_spmd(nc, in_maps, list(range(NCORES)),
                               trace=_want_trace)
    out = np.concatenate([res.results[c]["out"] for c in range(NCORES)],
                         axis=0).reshape(B, S, D)
    if _want_trace:
        return out, res
    return out


if __name__ == "__main__":
    rng = np.random.default_rng(0)
    x = rng.standard_normal((B, S, D), dtype=np.float32) * 0.1
    Wq = rng.standard_normal((D, NH * HD), dtype=np.float32) * 0.02
    Wk = rng.standard_normal((D, NKV * HD), dtype=np.float32) * 0.02
    Wv = rng.standard_normal((D, NKV * HD), dtype=np.float32) * 0.02
    Wo = rng.standard_normal((NH * HD, D), dtype=np.float32) * 0.02
    fc = rng.standard_normal((S, 64), dtype=np.float32)
    fs = rng.standard_normal((S, 64), dtype=np.float32)
    mask = np.where(np.tril(np.ones((S, S), bool)), 0.0,
                    np.float32(NEG_INF)).astype(np.float32)
    out = kernel(x, Wq, Wk, Wv, Wo, fc, fs, mask, 0)
    print("out", out.shape, out.dtype, np.abs(out).mean())


# revision 27
# speedup vs baseline: 1.0221x; 1.0179x over previous
"""Distributed Trainium2 kernel for GQA attention (B=2, S=2048, D=4096,
32 q-heads / 8 kv-heads, HD=128, RoPE, additive causal mask) on 8
NeuronCores.

Sharding: tensor-parallel over heads (4 q-heads + 1 kv-head per core).

Phase 1 computes the QKV projections weights-stationary so q^T/k^T come
out directly in [feat, tok] layout (RoPE applied in that layout via a
partition pair-swap stream shuffle); v is rotated back to [tok, feat]
with PE transposes. Phase 2 computes scores transposed (k stationary,
q^T moving -> s^T tiles), uses a max-free softmax (exp straight out of
PSUM, causal mask added only on diagonal tiles), feeds e^T directly to
the pv matmul (no p transposes), and accumulates per-row softmax
denominators with a ones-stationary matmul. Unnormalized o^T plus the
f32 row sums ride two AllToAlls (after head pair 0/1 and 2/3). Phase 3
normalizes the received o^T by the broadcast reciprocal row sums and
runs the Wo GEMM for this core's 512-token slice, ordering the
contraction so the second AllToAll hides under the first half's
compute. All matmuls bf16 with f32 PSUM accumulation.
"""
import sys

sys.path.insert(0, "/opt/trn_rl_repo")

import math
from contextlib import ExitStack
import numpy as np
import ml_dtypes

import concourse.bass as bass
import concourse.tile as tile
from concourse import bacc, mybir
from concourse.bass_utils import run_bass_kernel_spmd
from concourse.masks import make_identity

F32 = mybir.dt.float32
BF16 = mybir.dt.bfloat16
AF = mybir.ActivationFunctionType
OP = mybir.AluOpType

NCORES = 8
B, S, D = 2, 2048, 4096
NH, NKV, HD = 32, 8, 128
QH = NH // NCORES          # 4 q-heads per core
TOK = B * S                # 4096
TB = TOK // 512            # 8 blocks of 512 tokens
TT = TOK // 128            # 32 token tiles
TSLICE = TOK // NCORES     # 512 tokens out per core
NEG_INF = -1e9
SWAP32 = [i ^ 1 for i in range(32)]   # even/odd partition pair swap


def _build(mask_mode: str):
    nc = bacc.Bacc("TRN2", target_bir_lowering=False, debug=False,
                   enable_asserts=False, num_devices=NCORES)

    xT_e = nc.dram_tensor("xT", [D, TOK], BF16, kind="ExternalInput")
    Wq_e = nc.dram_tensor("Wqkv", [D, (QH + 2) * HD], BF16,
                          kind="ExternalInput")
    Wo_e = nc.dram_tensor("Wo", [D, D], BF16, kind="ExternalInput")
    # RoPE tables in transposed layout [feat 128, pos]: cq/sq q-scaled,
    # sq/sk carry the parity sign (-sin on even, +sin on odd partitions).
    cq_e = nc.dram_tensor("cqT", [128, S], F32, kind="ExternalInput")
    sq_e = nc.dram_tensor("sqT", [128, S], F32, kind="ExternalInput")
    ck_e = nc.dram_tensor("ckT", [128, S], F32, kind="ExternalInput")
    sk_e = nc.dram_tensor("skT", [128, S], F32, kind="ExternalInput")
    if mask_mode == "causal":
        mask_e = nc.dram_tensor("masktri", [128, 128], F32,
                                kind="ExternalInput")
    elif mask_mode == "general":
        mask_e = nc.dram_tensor("maskT", [S, S], F32, kind="ExternalInput")
    else:
        mask_e = None
    bias_e = nc.dram_tensor("biasB", [128, 1], F32, kind="ExternalInput") \
        if mask_mode == "general" else None
    out_e = nc.dram_tensor("out", [TSLICE, D], F32, kind="ExternalOutput")

    with tile.TileContext(nc) as tc, ExitStack() as ctx:
        _body(ctx, tc, mask_mode, xT_e, Wq_e, Wo_e,
              cq_e, sq_e, ck_e, sk_e, mask_e, bias_e, out_e)
    nc.compile()
    return nc


def _body(ctx, tc, mask_mode, xT_e, Wq_e, Wo_e,
          cq_e, sq_e, ck_e, sk_e, mask_e, bias_e, out_e):
    nc = tc.nc
    causal = mask_mode == "causal"

    consts = ctx.enter_context(tc.tile_pool(name="consts", bufs=1))
    ident_b = consts.tile([128, 128], BF16)
    make_identity(nc, ident_b[:])
    ones_sb = consts.tile([128, 128], BF16)
    nc.vector.memset(ones_sb[:], 1.0)

    # persistent across phases
    kv_pool = ctx.enter_context(tc.tile_pool(name="kv", bufs=1))
    qT_sb = kv_pool.tile([128, QH, TB, 512], BF16)  # [hd, h, tb, tok]
    kT_sb = kv_pool.tile([128, TB, 512], BF16)      # [hd, tb, tok]
    v_sb = kv_pool.tile([128, TT, 128], BF16)       # [tok, kt, feat]

    dram = ctx.enter_context(tc.tile_pool(name="dram", bufs=1, space="DRAM"))
    # per slot: 256 rows of o^T (2 heads) + 4 rows = 2x[1,512] f32 rowsums
    a2a_in1 = dram.tile([NCORES, 260, TSLICE], BF16)
    a2a_out1 = dram.tile([NCORES, 260, TSLICE], BF16)
    a2a_in2 = dram.tile([NCORES, 260, TSLICE], BF16)
    a2a_out2 = dram.tile([NCORES, 260, TSLICE], BF16)
    rdram = dram.tile([2, 16, 512], F32)

    # ---------------- Phase 1: QKV projections + RoPE --------------------
    with tc.tile_pool(name="pj_w", bufs=1) as wpool, \
         tc.tile_pool(name="pj_x", bufs=2) as xpool, \
         tc.tile_pool(name="pj_rt", bufs=1) as rtpool, \
         tc.tile_pool(name="pj_qs", bufs=2) as qspool, \
         tc.tile_pool(name="pj_tmp", bufs=1) as tmppool, \
         tc.tile_pool(name="pj_st", bufs=3) as stpool, \
         tc.tile_pool(name="pj_psq", bufs=1, space="PSUM") as psq_pool, \
         tc.tile_pool(name="pj_pskv", bufs=1, space="PSUM") as pskv_pool, \
         tc.tile_pool(name="pj_pstr", bufs=2, space="PSUM") as pstr_pool:

        Wq_sb = wpool.tile([128, 32, (QH + 2) * HD], BF16)
        wq_re = Wq_e.ap().rearrange("(k p) f -> p k f", p=128)
        # split the weight load so the first k-slices land fast
        nc.sync.dma_start(out=Wq_sb[:, 0:1, :], in_=wq_re[:, 0:1, :])
        nc.sync.dma_start(out=Wq_sb[:, 1:4, :], in_=wq_re[:, 1:4, :])
        nc.sync.dma_start(out=Wq_sb[:, 4:16, :], in_=wq_re[:, 4:16, :])
        nc.sync.dma_start(out=Wq_sb[:, 16:32, :], in_=wq_re[:, 16:32, :])

        ctabs = {}
        for nm, te in (("cq", cq_e), ("sq", sq_e), ("ck", ck_e), ("sk", sk_e)):
            t = rtpool.tile([128, 4, 512], F32, name=f"tab_{nm}",
                            tag=f"tab_{nm}")
            nc.scalar.dma_start(
                out=t[:], in_=te.ap().rearrange("p (sb t) -> p sb t", t=512))
            ctabs[nm] = t

        def rep2(ap):  # repeat a [128, 512] table slice 2x along free dim
            return bass.AP(tensor=ap.tensor, offset=ap.offset,
                           ap=[ap.ap[0], [0, 2], ap.ap[-1]])

        for tb in range(TB):
            sblk = tb % 4   # position block within batch
            qps = psq_pool.tile([128, QH, 512], F32)
            kvps = pskv_pool.tile([128, 2, 512], F32)
            for kq in range(4):
                xsl = xpool.tile([128, 8, 512], BF16)
                xsrc = xT_e.ap()[:, tb * 512:(tb + 1) * 512] \
                    .rearrange("(k p) t -> p k t", p=128)
                if tb == 0 and kq == 0:
                    # split the very first load so matmuls start sooner
                    nc.gpsimd.dma_start(out=xsl[:, 0:2, :],
                                        in_=xsrc[:, 0:2, :])
                    nc.gpsimd.dma_start(out=xsl[:, 2:8, :],
                                        in_=xsrc[:, 2:8, :])
                else:
                    nc.sync.dma_start(out=xsl[:],
                                      in_=xsrc[:, kq * 8:(kq + 1) * 8, :])
                for k in range(8):
                    kk = kq * 8 + k
                    st = (kk == 0)
                    sp = (kk == 31)
                    # kv first: their banks free soonest at block turnover
                    for fc in range(2):
                        nc.tensor.matmul(
                            kvps[:, fc, :],
                            Wq_sb[:, kk, (QH + fc) * 128:(QH + fc + 1) * 128],
                            xsl[:, k, :], start=st, stop=sp)
                    for fc in range(QH):
                        nc.tensor.matmul(
                            qps[:, fc, :],
                            Wq_sb[:, kk, fc * 128:(fc + 1) * 128],
                            xsl[:, k, :], start=st, stop=sp)

            # ---- q: evict psum fast (scalar, split so the next block's
            # first accumulations can reuse the early banks sooner)
            qpre = qspool.tile([128, QH, 512], BF16)
            for fc in range(QH):
                nc.scalar.copy(qpre[:, fc, :], qps[:, fc, :])
            for hp in range(2):
                qsl = qpre[:, hp * 2:(hp + 1) * 2, :]
                qshuf = tmppool.tile([128, 2, 512], BF16, name="qshuf",
                                     tag="qshuf", bufs=2)
                nc.vector.stream_shuffle(qshuf[:], qsl, SWAP32)
                t1 = tmppool.tile([128, 2, 512], F32, name="t1", tag="t1",
                                  bufs=2)
                t2 = tmppool.tile([128, 2, 512], F32, name="t2", tag="t2",
                                  bufs=2)
                nc.vector.tensor_mul(t1[:], qsl, rep2(ctabs["cq"][:, sblk, :]))
                nc.vector.tensor_mul(t2[:], qshuf[:],
                                     rep2(ctabs["sq"][:, sblk, :]))
                nc.vector.tensor_add(qT_sb[:, hp * 2:(hp + 1) * 2, tb, :],
                                     t1[:], t2[:])

            # ---- k: evict psum fast (scalar), then RoPE on vector
            kpre = qspool.tile([128, 512], BF16, name="kpre", tag="kpre",
                               bufs=2)
            nc.scalar.copy(kpre[:], kvps[:, 0, :])
            kshuf = tmppool.tile([128, 512], BF16, name="kshuf", tag="kshuf",
                                 bufs=2)
            nc.vector.stream_shuffle(kshuf[:], kpre[:], SWAP32)
            u1 = tmppool.tile([128, 512], F32, name="u1", tag="u1", bufs=2)
            u2 = tmppool.tile([128, 512], F32, name="u2", tag="u2", bufs=2)
            nc.vector.tensor_mul(u1[:], kpre[:], ctabs["ck"][:, sblk, :])
            nc.vector.tensor_mul(u2[:], kshuf[:], ctabs["sk"][:, sblk, :])
            nc.vector.tensor_add(kT_sb[:, tb, :], u1[:], u2[:])

            # ---- v: evict, transpose back to [tok, feat]
            vT_st = stpool.tile([128, 512], BF16)
            nc.scalar.copy(vT_st[:], kvps[:, 1, :])
            vtp = pstr_pool.tile([128, 512], BF16)
            for i in range(4):
                nc.tensor.transpose(vtp[:, i * 128:(i + 1) * 128],
                                    vT_st[:, i * 128:(i + 1) * 128],
                                    ident_b[:])
            nc.scalar.copy(v_sb[:, tb * 4:(tb + 1) * 4, :], vtp[:])

    # ------- pool for o^T receive, placed over dead Phase-1 space --------
    p3keep = ctx.enter_context(tc.tile_pool(name="p3keep", bufs=1))
    oT_sb = p3keep.tile([128, NH, TSLICE], BF16)   # [feat, hg, tok]
    # f32 rowsums: group 1 on partitions 0-15, group 2 on 32-47 (legal
    # base partitions for the batched reciprocal)
    rraw = p3keep.tile([64, 512], F32)
    rinv = p3keep.tile([64, 512], F32)

    def rbt_tile():
        return p3keep.tile([128, 512], F32, name="rbt", tag="rbt", bufs=4)

    # first Wo tiles of GEMM pass 1, prefetched while P2 finishes
    wot_pre = p3keep.tile([128, 8, 512], BF16)

    # ---------------- Phase 2: attention per (head, batch, qtg) ----------
    with tc.tile_pool(name="at_z", bufs=1) as zpool, \
         tc.tile_pool(name="at_et", bufs=2) as etpool, \
         tc.tile_pool(name="at_m", bufs=4) as mpool, \
         tc.tile_pool(name="at_o", bufs=6) as opool, \
         tc.tile_pool(name="at_ps_s", bufs=5, space="PSUM") as spsum, \
         tc.tile_pool(name="at_ps_o", bufs=2, space="PSUM") as opsum, \
         tc.tile_pool(name="at_ps_r", bufs=1, space="PSUM") as rpsum:

        if causal:
            # [128,128] upper-triangle additive mask (0 if p<=c else -1e9)
            masktri_sb = zpool.tile([128, 128], F32)
            nc.gpsimd.dma_start(out=masktri_sb[:], in_=mask_e.ap())
        if mask_mode == "general":
            biasB_sb = zpool.tile([128, 1], F32)
            nc.gpsimd.dma_start(out=biasB_sb[:], in_=bias_e.ap())

        def attn_block(h, b, qtg):
            nkt = 4 * (qtg + 1) if causal else 16
            d = b * 4 + qtg
            eT = etpool.tile([128, 16, 512], BF16)
            otps = opsum.tile([128, 512], F32)
            rps = rpsum.tile([128, 512], F32)

            def col0(kt):
                if causal and kt // 4 == qtg:
                    return (kt % 4) * 128
                return 0

            def s_mm(kt):
                c0 = col0(kt)
                tbk = b * 4 + kt // 4
                sps = spsum.tile([128, 512], F32, name="sps", tag="sps")
                nc.tensor.matmul(
                    sps[:, c0:512],
                    kT_sb[:, tbk, (kt % 4) * 128:(kt % 4 + 1) * 128],
                    qT_sb[:, h, b * 4 + qtg, c0:512],
                    start=True, stop=True)
                if causal and kt // 4 == qtg:
                    nc.vector.tensor_add(sps[:, c0:c0 + 128],
                                         sps[:, c0:c0 + 128], masktri_sb[:])
                elif mask_mode == "general":
                    mt = mpool.tile([128, 512], F32)
                    nc.gpsimd.dma_start(
                        out=mt[:],
                        in_=mask_e.ap()[kt * 128:(kt + 1) * 128,
                                        qtg * 512:(qtg + 1) * 512])
                    nc.vector.tensor_add(sps[:], sps[:], mt[:])
                bias = biasB_sb[:] if mask_mode == "general" else 0.0
                nc.scalar.activation(eT[:, kt, c0:512], sps[:, c0:512],
                                     AF.Exp, bias=bias, scale=1.0)

            LA = 5
            for kt in range(min(LA, nkt)):
                s_mm(kt)
            for kt in range(nkt):
                if kt + LA < nkt:
                    s_mm(kt + LA)
                c0 = col0(kt)
                nc.tensor.matmul(otps[:, c0:512], v_sb[:, b * 16 + kt, :],
                                 eT[:, kt, c0:512], start=(kt == 0),
                                 stop=(kt == nkt - 1))
                nc.tensor.matmul(rps[:, c0:512], ones_sb[:],
                                 eT[:, kt, c0:512], start=(kt == 0),
                                 stop=(kt == nkt - 1))

            osb = opool.tile([128, 512], BF16, name="osb", tag="osb")
            nc.vector.tensor_copy(osb[:], otps[:])
            rsb = opool.tile([1, 512], F32, name="rsb", tag="rsb")
            nc.vector.tensor_copy(rsb[:], rps[0:1, :])
            tgt = a2a_in1 if h < 2 else a2a_in2
            hl = h % 2
            nc.sync.dma_start(out=tgt[d, hl * 128:(hl + 1) * 128, :],
                              in_=osb[:])
            nc.sync.dma_start(out=tgt[d, 256 + hl * 2:258 + hl * 2, :],
                              in_=rsb[:].bitcast(BF16))

        for h in range(QH):
            for b in range(B):
                for qtg in range(4):
                    attn_block(h, b, qtg)
            if h == 1:
                # gpsimd queue is otherwise empty in P2, so the trigger
                # waiting at its head costs nothing; completes during
                # head-2/3 compute and the receive below hides too
                nc.gpsimd.collective_compute(
                    "AllToAll", OP.bypass,
                    replica_groups=[list(range(NCORES))],
                    ins=[a2a_in1.opt()], outs=[a2a_out1.opt()])

        # group-1 o^T + rowsums land while heads 2/3 still compute
        for s in range(NCORES):
            for hl in range(2):
                nc.gpsimd.dma_start(
                    out=oT_sb[:, s * QH + hl, :],
                    in_=a2a_out1[s, hl * 128:(hl + 1) * 128, :])
                nc.gpsimd.dma_start(
                    out=rraw[s * 2 + hl:s * 2 + hl + 1, :],
                    in_=a2a_out1[s, 256 + hl * 2:258 + hl * 2, :]
                    .bitcast(F32))
        nc.vector.reciprocal(rinv[0:16, :], rraw[0:16, :])
        nc.sync.dma_start(out=rdram[0], in_=rinv[0:16, :])
        # broadcast 1/r over partitions via stride-0 DRAM reads, then
        # normalize group-1 o^T in place
        for s in range(NCORES):
            for hl in range(2):
                hg = s * QH + hl
                rbt = rbt_tile()
                nc.gpsimd.dma_start(
                    out=rbt[:],
                    in_=bass.AP(tensor=rdram.tensor,
                                offset=rdram.offset + (s * 2 + hl) * 512,
                                ap=[[0, 128], [1, 512]]))
                # gpsimd: the vector queue is saturated with P2 mask adds
                nc.gpsimd.tensor_mul(oT_sb[:, hg, :], oT_sb[:, hg, :],
                                     rbt[:])

        # prefetch the first 8 pass-1 Wo tiles (dc=0) on the idle gpsimd
        # queue so the GEMM is fed the moment P2's last exp drains
        grp1_order = [s * QH + g for s in range(NCORES) for g in (0, 1)]
        for j in range(8):
            hg = grp1_order[j]
            nc.gpsimd.dma_start(
                out=wot_pre[:, j, :],
                in_=Wo_e.ap()[hg * 128:(hg + 1) * 128, 0:512])

        nc.gpsimd.collective_compute(
            "AllToAll", OP.bypass,
            replica_groups=[list(range(NCORES))],
            ins=[a2a_in2.opt()], outs=[a2a_out2.opt()])

    # ---------------- Phase 3: Wo GEMM on own token slice ----------------
    with tc.tile_pool(name="wo_acc", bufs=1) as accpool, \
         tc.tile_pool(name="wo_w", bufs=20) as wopool, \
         tc.tile_pool(name="wo_out", bufs=6) as outpool, \
         tc.tile_pool(name="wo_ps", bufs=2, space="PSUM") as wopsum:

        for s in range(NCORES):
            for hl in range(2):
                nc.gpsimd.dma_start(
                    out=oT_sb[:, s * QH + 2 + hl, :],
                    in_=a2a_out2[s, hl * 128:(hl + 1) * 128, :])
                nc.gpsimd.dma_start(
                    out=rraw[32 + s * 2 + hl:32 + s * 2 + hl + 1, :],
                    in_=a2a_out2[s, 256 + hl * 2:258 + hl * 2, :]
                    .bitcast(F32))
        nc.vector.reciprocal(rinv[32:48, :], rraw[32:48, :])
        nc.sync.dma_start(out=rdram[1], in_=rinv[32:48, :])
        for s in range(NCORES):
            for hl in range(2):
                hg = s * QH + 2 + hl
                rbt = rbt_tile()
                dq = (nc.gpsimd, nc.scalar, nc.sync)[(s * 2 + hl) % 3]
                dq.dma_start(
                    out=rbt[:],
                    in_=bass.AP(tensor=rdram.tensor,
                                offset=rdram.offset + (16 + s * 2 + hl) * 512,
                                ap=[[0, 128], [1, 512]]))
                nc.vector.tensor_mul(oT_sb[:, hg, :], oT_sb[:, hg, :],
                                     rbt[:])

        grp1 = [s * QH + g for s in range(NCORES) for g in (0, 1)]
        grp2 = [s * QH + 2 + g for s in range(NCORES) for g in (0, 1)]
        accs = []
        # pass 1: group-1 contraction only -> SBUF accumulators, giving
        # the second AllToAll a full half-GEMM to hide under
        for dc in range(8):
            pso = wopsum.tile([128, 4, 512], F32, name="pso", tag="pso")
            for ci, hg in enumerate(grp1):
                if dc == 0 and ci < 8:
                    wv = wot_pre[:, ci, :]
                else:
                    wot = wopool.tile([128, 512], BF16)
                    dq = nc.scalar if ci % 2 == 0 else nc.sync
                    dq.dma_start(out=wot[:],
                                 in_=Wo_e.ap()[hg * 128:(hg + 1) * 128,
                                               dc * 512:(dc + 1) * 512])
                    wv = wot[:]
                for t in range(4):
                    nc.tensor.matmul(pso[:, t, :],
                                     oT_sb[:, hg, t * 128:(t + 1) * 128],
                                     wv, start=(ci == 0), stop=(ci == 15))
            acc = accpool.tile([128, 4, 512], F32, name=f"acc{dc}",
                               tag=f"acc{dc}")
            accs.append(acc)
            nc.vector.tensor_copy(acc[:], pso[:])
        # pass 2: group-2 contraction + partial sums -> output
        for dc in range(8):
            pso = wopsum.tile([128, 4, 512], F32, name="pso", tag="pso")
            for ci, hg in enumerate(grp2):
                wot = wopool.tile([128, 512], BF16)
                dq = (nc.gpsimd, nc.scalar, nc.sync)[ci % 3]
                dq.dma_start(out=wot[:],
                             in_=Wo_e.ap()[hg * 128:(hg + 1) * 128,
                                           dc * 512:(dc + 1) * 512])
                for t in range(4):
                    nc.tensor.matmul(pso[:, t, :],
                                     oT_sb[:, hg, t * 128:(t + 1) * 128],
                                     wot[:], start=(ci == 0), stop=(ci == 15))
            for t in range(4):
                osb = outpool.tile([128, 512], F32)
                nc.vector.tensor_add(osb[:], accs[dc][:, t, :], pso[:, t, :])
                nc.sync.dma_start(
                    out=out_e.ap()[t * 128:(t + 1) * 128,
                                   dc * 512:(dc + 1) * 512],
                    in_=osb[:])


_NC_CACHE = {}


def _get_nc(mask_mode):
    if mask_mode not in _NC_CACHE:
        _NC_CACHE[mask_mode] = _build(mask_mode)
    return _NC_CACHE[mask_mode]


def _estimate_score_bound(x, Wq, Wk, fc, fs):
    """Sampled upper estimate of max |q.k/sqrt(hd)| after RoPE."""
    rng = np.random.default_rng(12345)
    x2 = x.reshape(TOK, D)
    rq = rng.choice(TOK, 192, replace=False)
    rk = rng.choice(TOK, 192, replace=False)
    q = (x2[rq] @ Wq).reshape(192, NH, HD)
    k = (x2[rk] @ Wk).reshape(192, NKV, HD)

    def rope(t, pos):
        c, s = fc[pos % S], fs[pos % S]
        tr, ti = t[..., 0::2], t[..., 1::2]
        out = np.empty_like(t)
        out[..., 0::2] = tr * c[:, None, :] - ti * s[:, None, :]
        out[..., 1::2] = tr * s[:, None, :] + ti * c[:, None, :]
        return out

    q = rope(q, rq)
    k = rope(k, rk)
    qg = q.reshape(192, NKV, NH // NKV, HD)
    sc = np.einsum('qgnd,kgd->gnqk', qg, k) / np.float32(math.sqrt(HD))
    return float(np.abs(sc).max())


def _rope_tables(freqs_cos, freqs_sin, scale):
    """[128, S] transposed tables: c duplicated on partition pairs,
    s with -sin on even / +sin on odd partitions."""
    c = np.empty((128, S), np.float32)
    s = np.empty((128, S), np.float32)
    c[0::2] = c[1::2] = freqs_cos.T * scale
    s[0::2] = -freqs_sin.T * scale
    s[1::2] = freqs_sin.T * scale
    return np.ascontiguousarray(c), np.ascontiguousarray(s)


def kernel(x, Wq, Wk, Wv, Wo, freqs_cos, freqs_sin, mask, start_pos=0,
           _want_trace=False):
    x = np.asarray(x, dtype=np.float32)
    mask = np.asarray(mask, dtype=np.float32)
    freqs_cos = np.asarray(freqs_cos, dtype=np.float32)
    freqs_sin = np.asarray(freqs_sin, dtype=np.float32)

    if not mask.any():
        mask_mode = "zeros"
    else:
        canon = np.where(np.tril(np.ones((S, S), bool)), 0.0,
                         np.float32(NEG_INF)).astype(np.float32)
        mask_mode = "causal" if np.array_equal(mask, canon) else "general"

    xT = np.ascontiguousarray(x.reshape(TOK, D).T.astype(ml_dtypes.bfloat16))
    scale = np.float32(1.0 / math.sqrt(HD))
    cqT, sqT = _rope_tables(freqs_cos, freqs_sin, scale)
    ckT, skT = _rope_tables(freqs_cos, freqs_sin, np.float32(1.0))
    Wo_bf = np.ascontiguousarray(np.asarray(Wo, np.float32)
                                 .astype(ml_dtypes.bfloat16))
    if mask_mode == "causal":
        # [128,128] upper-triangle additive mask in transposed layout
        p = np.arange(128)[:, None]
        q = np.arange(128)[None, :]
        masktri = np.where(p <= q, 0.0, NEG_INF).astype(np.float32)
        masktri = np.ascontiguousarray(masktri)

    in_maps = []
    for c in range(NCORES):
        m = {
            "xT": xT,
            "Wqkv": np.ascontiguousarray(np.concatenate(
                [Wq[:, c * QH * HD:(c + 1) * QH * HD],
                 Wk[:, c * HD:(c + 1) * HD],
                 Wv[:, c * HD:(c + 1) * HD]],
                axis=1).astype(ml_dtypes.bfloat16)),
            "Wo": Wo_bf,
            "cqT": cqT, "sqT": sqT, "ckT": ckT, "skT": skT,
        }
        if mask_mode == "causal":
            m["masktri"] = masktri
        elif mask_mode == "general":
            m["maskT"] = np.ascontiguousarray(mask.T)
            bound = _estimate_score_bound(x, Wq, Wk, freqs_cos, freqs_sin)
            bmax = float(np.max(mask[np.isfinite(mask)])) if \
                np.isfinite(mask).any() else 0.0
            m["biasB"] = np.full((128, 1), -(1.25 * bound + max(bmax, 0.0)),
                                 np.float32)
        in_maps.append(m)

    nc = _get_nc(mask_mode)
    res = run_bass_kernel_spmd(nc, in_maps, list(range(NCORES)),
                               trace=_want_trace)
    out = np.concatenate([res.results[c]["out"] for c in range(NCORES)],
                         axis=0).reshape(B, S, D)
    if _want_trace:
        return out, res
    return out


if __name__ == "__main__":
    rng = np.random.default_rng(0)
    x = rng.standard_normal((B, S, D), dtype=np.float32) * 0.1
    Wq = rng.standard_normal((D, NH * HD), dtype=np.float32) * 0.02
    Wk = rng.standard_normal((D, NKV * HD), dtype=np.float32) * 0.02
    Wv = rng.standard_normal((D, NKV * HD), dtype=np.float32) * 0.02
    Wo = rng.standard_normal((NH * HD, D), dtype=np.float32) * 0.02
    fc = rng.standard_normal((S, 64), dtype=np.float32)
    fs = rng.standard_normal((S, 64), dtype=np.float32)
    mask = np.where(np.tril(np.ones((S, S), bool)), 0.0,
                    np.float32(NEG_INF)).astype(np.float32)
    out = kernel(x, Wq, Wk, Wv, Wo, fc, fs, mask, 0)
    print("out", out.shape, out.dtype, np.abs(out).mean())


# revision 28
# speedup vs baseline: 1.0333x; 1.0109x over previous
"""Distributed Trainium2 kernel for GQA attention (B=2, S=2048, D=4096,
32 q-heads / 8 kv-heads, HD=128, RoPE, additive causal mask) on 8
NeuronCores.

Sharding: tensor-parallel over heads (4 q-heads + 1 kv-head per core).

Phase 1 computes the QKV projections weights-stationary so q^T/k^T come
out directly in [feat, tok] layout (RoPE applied in that layout via a
partition pair-swap stream shuffle); v is rotated back to [tok, feat]
with PE transposes. Phase 2 computes scores transposed (k stationary,
q^T moving -> s^T tiles), uses a max-free softmax (exp straight out of
PSUM, causal mask added only on diagonal tiles), feeds e^T directly to
the pv matmul (no p transposes), and accumulates per-row softmax
denominators with a ones-stationary matmul. Unnormalized o^T plus the
f32 row sums ride two AllToAlls (after head pair 0/1 and 2/3). Phase 3
normalizes the received o^T by the broadcast reciprocal row sums and
runs the Wo GEMM for this core's 512-token slice, ordering the
contraction so the second AllToAll hides under the first half's
compute. All matmuls bf16 with f32 PSUM accumulation.
"""
import sys

sys.path.insert(0, "/opt/trn_rl_repo")

import math
from contextlib import ExitStack
import numpy as np
import ml_dtypes

import concourse.bass as bass
import concourse.tile as tile
from concourse import bacc, mybir
from concourse.bass_utils import run_bass_kernel_spmd
from concourse.masks import make_identity

F32 = mybir.dt.float32
BF16 = mybir.dt.bfloat16
AF = mybir.ActivationFunctionType
OP = mybir.AluOpType

NCORES = 8
B, S, D = 2, 2048, 4096
NH, NKV, HD = 32, 8, 128
QH = NH // NCORES          # 4 q-heads per core
TOK = B * S                # 4096
TB = TOK // 512            # 8 blocks of 512 tokens
TT = TOK // 128            # 32 token tiles
TSLICE = TOK // NCORES     # 512 tokens out per core
NEG_INF = -1e9
SWAP32 = [i ^ 1 for i in range(32)]   # even/odd partition pair swap


def _build(mask_mode: str):
    nc = bacc.Bacc("TRN2", target_bir_lowering=False, debug=False,
                   enable_asserts=False, num_devices=NCORES)

    xT_e = nc.dram_tensor("xT", [D, TOK], BF16, kind="ExternalInput")
    Wq_e = nc.dram_tensor("Wqkv", [D, (QH + 2) * HD], BF16,
                          kind="ExternalInput")
    Wo_e = nc.dram_tensor("Wo", [D, D], BF16, kind="ExternalInput")
    # RoPE tables in transposed layout [feat 128, pos]: cq/sq q-scaled,
    # sq/sk carry the parity sign (-sin on even, +sin on odd partitions).
    cq_e = nc.dram_tensor("cqT", [128, S], F32, kind="ExternalInput")
    sq_e = nc.dram_tensor("sqT", [128, S], F32, kind="ExternalInput")
    ck_e = nc.dram_tensor("ckT", [128, S], F32, kind="ExternalInput")
    sk_e = nc.dram_tensor("skT", [128, S], F32, kind="ExternalInput")
    if mask_mode == "causal":
        mask_e = nc.dram_tensor("masktri", [128, 128], F32,
                                kind="ExternalInput")
    elif mask_mode == "general":
        mask_e = nc.dram_tensor("maskT", [S, S], F32, kind="ExternalInput")
    else:
        mask_e = None
    bias_e = nc.dram_tensor("biasB", [128, 1], F32, kind="ExternalInput") \
        if mask_mode == "general" else None
    out_e = nc.dram_tensor("out", [TSLICE, D], F32, kind="ExternalOutput")

    with tile.TileContext(nc) as tc, ExitStack() as ctx:
        _body(ctx, tc, mask_mode, xT_e, Wq_e, Wo_e,
              cq_e, sq_e, ck_e, sk_e, mask_e, bias_e, out_e)
    nc.compile()
    return nc


def _body(ctx, tc, mask_mode, xT_e, Wq_e, Wo_e,
          cq_e, sq_e, ck_e, sk_e, mask_e, bias_e, out_e):
    nc = tc.nc
    causal = mask_mode == "causal"

    consts = ctx.enter_context(tc.tile_pool(name="consts", bufs=1))
    ident_b = consts.tile([128, 128], BF16)
    make_identity(nc, ident_b[:])
    ones_sb = consts.tile([128, 128], BF16)
    nc.vector.memset(ones_sb[:], 1.0)

    # persistent across phases
    kv_pool = ctx.enter_context(tc.tile_pool(name="kv", bufs=1))
    qT_sb = kv_pool.tile([128, QH, TB, 512], BF16)  # [hd, h, tb, tok]
    kT_sb = kv_pool.tile([128, TB, 512], BF16)      # [hd, tb, tok]
    v_sb = kv_pool.tile([128, TT, 128], BF16)       # [tok, kt, feat]

    dram = ctx.enter_context(tc.tile_pool(name="dram", bufs=1, space="DRAM"))
    # per slot: 256 rows of o^T (2 heads) + 4 rows = 2x[1,512] f32 rowsums
    a2a_in1 = dram.tile([NCORES, 260, TSLICE], BF16)
    a2a_out1 = dram.tile([NCORES, 260, TSLICE], BF16)
    a2a_in2 = dram.tile([NCORES, 260, TSLICE], BF16)
    a2a_out2 = dram.tile([NCORES, 260, TSLICE], BF16)
    rdram = dram.tile([2, 16, 512], F32)

    # ---------------- Phase 1: QKV projections + RoPE --------------------
    with tc.tile_pool(name="pj_w", bufs=1) as wpool, \
         tc.tile_pool(name="pj_x", bufs=2) as xpool, \
         tc.tile_pool(name="pj_rt", bufs=1) as rtpool, \
         tc.tile_pool(name="pj_qs", bufs=2) as qspool, \
         tc.tile_pool(name="pj_tmp", bufs=1) as tmppool, \
         tc.tile_pool(name="pj_st", bufs=3) as stpool, \
         tc.tile_pool(name="pj_psq", bufs=1, space="PSUM") as psq_pool, \
         tc.tile_pool(name="pj_pskv", bufs=1, space="PSUM") as pskv_pool, \
         tc.tile_pool(name="pj_pstr", bufs=2, space="PSUM") as pstr_pool:

        Wq_sb = wpool.tile([128, 32, (QH + 2) * HD], BF16)
        wq_re = Wq_e.ap().rearrange("(k p) f -> p k f", p=128)
        # split the weight load so the first k-slices land fast
        nc.sync.dma_start(out=Wq_sb[:, 0:1, :], in_=wq_re[:, 0:1, :])
        nc.sync.dma_start(out=Wq_sb[:, 1:4, :], in_=wq_re[:, 1:4, :])
        # tails on scalar: the sync queue carries the xsl stream and the
        # big weight loads would delay tb0's later k-quarters
        nc.scalar.dma_start(out=Wq_sb[:, 4:16, :], in_=wq_re[:, 4:16, :])
        nc.scalar.dma_start(out=Wq_sb[:, 16:32, :], in_=wq_re[:, 16:32, :])

        ctabs = {}
        for nm, te in (("cq", cq_e), ("sq", sq_e), ("ck", ck_e), ("sk", sk_e)):
            t = rtpool.tile([128, 4, 512], F32, name=f"tab_{nm}",
                            tag=f"tab_{nm}")
            nc.scalar.dma_start(
                out=t[:], in_=te.ap().rearrange("p (sb t) -> p sb t", t=512))
            ctabs[nm] = t

        def rep2(ap):  # repeat a [128, 512] table slice 2x along free dim
            return bass.AP(tensor=ap.tensor, offset=ap.offset,
                           ap=[ap.ap[0], [0, 2], ap.ap[-1]])

        for tb in range(TB):
            sblk = tb % 4   # position block within batch
            qps = psq_pool.tile([128, QH, 512], F32)
            kvps = pskv_pool.tile([128, 2, 512], F32)
            for kq in range(4):
                xsl = xpool.tile([128, 8, 512], BF16)
                xsrc = xT_e.ap()[:, tb * 512:(tb + 1) * 512] \
                    .rearrange("(k p) t -> p k t", p=128)
                if tb == 0 and kq == 0:
                    # split the very first load so matmuls start sooner
                    nc.gpsimd.dma_start(out=xsl[:, 0:2, :],
                                        in_=xsrc[:, 0:2, :])
                    nc.gpsimd.dma_start(out=xsl[:, 2:8, :],
                                        in_=xsrc[:, 2:8, :])
                else:
                    nc.sync.dma_start(out=xsl[:],
                                      in_=xsrc[:, kq * 8:(kq + 1) * 8, :])
                for k in range(8):
                    kk = kq * 8 + k
                    st = (kk == 0)
                    sp = (kk == 31)
                    # kv first: their banks free soonest at block turnover
                    for fc in range(2):
                        nc.tensor.matmul(
                            kvps[:, fc, :],
                            Wq_sb[:, kk, (QH + fc) * 128:(QH + fc + 1) * 128],
                            xsl[:, k, :], start=st, stop=sp)
                    for fc in range(QH):
                        nc.tensor.matmul(
                            qps[:, fc, :],
                            Wq_sb[:, kk, fc * 128:(fc + 1) * 128],
                            xsl[:, k, :], start=st, stop=sp)

            # ---- k/v evictions first: next block's matmuls hit the kv
            # banks first, so free those before the q banks
            kpre = qspool.tile([128, 512], BF16, name="kpre", tag="kpre",
                               bufs=2)
            nc.scalar.copy(kpre[:], kvps[:, 0, :])
            vT_st = stpool.tile([128, 512], BF16)
            nc.scalar.copy(vT_st[:], kvps[:, 1, :])
            qpre = qspool.tile([128, QH, 512], BF16)
            for fc in range(QH):
                nc.scalar.copy(qpre[:, fc, :], qps[:, fc, :])
            for hp in range(2):
                qsl = qpre[:, hp * 2:(hp + 1) * 2, :]
                qshuf = tmppool.tile([128, 2, 512], BF16, name="qshuf",
                                     tag="qshuf", bufs=2)
                nc.vector.stream_shuffle(qshuf[:], qsl, SWAP32)
                t1 = tmppool.tile([128, 2, 512], F32, name="t1", tag="t1",
                                  bufs=2)
                t2 = tmppool.tile([128, 2, 512], F32, name="t2", tag="t2",
                                  bufs=2)
                nc.vector.tensor_mul(t1[:], qsl, rep2(ctabs["cq"][:, sblk, :]))
                nc.vector.tensor_mul(t2[:], qshuf[:],
                                     rep2(ctabs["sq"][:, sblk, :]))
                nc.vector.tensor_add(qT_sb[:, hp * 2:(hp + 1) * 2, tb, :],
                                     t1[:], t2[:])

            # ---- k: RoPE on vector from the early eviction
            kshuf = tmppool.tile([128, 512], BF16, name="kshuf", tag="kshuf",
                                 bufs=2)
            nc.vector.stream_shuffle(kshuf[:], kpre[:], SWAP32)
            u1 = tmppool.tile([128, 512], F32, name="u1", tag="u1", bufs=2)
            u2 = tmppool.tile([128, 512], F32, name="u2", tag="u2", bufs=2)
            nc.vector.tensor_mul(u1[:], kpre[:], ctabs["ck"][:, sblk, :])
            nc.vector.tensor_mul(u2[:], kshuf[:], ctabs["sk"][:, sblk, :])
            nc.vector.tensor_add(kT_sb[:, tb, :], u1[:], u2[:])

            # ---- v: transpose back to [tok, feat]
            vtp = pstr_pool.tile([128, 512], BF16)
            for i in range(4):
                nc.tensor.transpose(vtp[:, i * 128:(i + 1) * 128],
                                    vT_st[:, i * 128:(i + 1) * 128],
                                    ident_b[:])
            nc.scalar.copy(v_sb[:, tb * 4:(tb + 1) * 4, :], vtp[:])

    # ------- pool for o^T receive, placed over dead Phase-1 space --------
    p3keep = ctx.enter_context(tc.tile_pool(name="p3keep", bufs=1))
    oT_sb = p3keep.tile([128, NH, TSLICE], BF16)   # [feat, hg, tok]
    # f32 rowsums: group 1 on partitions 0-15, group 2 on 32-47 (legal
    # base partitions for the batched reciprocal)
    rraw = p3keep.tile([64, 512], F32)
    rinv = p3keep.tile([64, 512], F32)

    def rbt_tile():
        return p3keep.tile([128, 512], F32, name="rbt", tag="rbt", bufs=4)

    # first Wo tiles of GEMM pass 1, prefetched while P2 finishes
    wot_pre = p3keep.tile([128, 8, 512], BF16)

    # ---------------- Phase 2: attention per (head, batch, qtg) ----------
    with tc.tile_pool(name="at_z", bufs=1) as zpool, \
         tc.tile_pool(name="at_et", bufs=2) as etpool, \
         tc.tile_pool(name="at_m", bufs=4) as mpool, \
         tc.tile_pool(name="at_o", bufs=6) as opool, \
         tc.tile_pool(name="at_ps_s", bufs=5, space="PSUM") as spsum, \
         tc.tile_pool(name="at_ps_o", bufs=2, space="PSUM") as opsum, \
         tc.tile_pool(name="at_ps_r", bufs=1, space="PSUM") as rpsum:

        if causal:
            # [128,128] upper-triangle additive mask (0 if p<=c else -1e9)
            masktri_sb = zpool.tile([128, 128], F32)
            nc.gpsimd.dma_start(out=masktri_sb[:], in_=mask_e.ap())
        if mask_mode == "general":
            biasB_sb = zpool.tile([128, 1], F32)
            nc.gpsimd.dma_start(out=biasB_sb[:], in_=bias_e.ap())

        def attn_block(h, b, qtg):
            nkt = 4 * (qtg + 1) if causal else 16
            d = b * 4 + qtg
            eT = etpool.tile([128, 16, 512], BF16)
            otps = opsum.tile([128, 512], F32)
            rps = rpsum.tile([128, 512], F32)

            def col0(kt):
                if causal and kt // 4 == qtg:
                    return (kt % 4) * 128
                return 0

            def s_mm(kt):
                c0 = col0(kt)
                tbk = b * 4 + kt // 4
                sps = spsum.tile([128, 512], F32, name="sps", tag="sps")
                nc.tensor.matmul(
                    sps[:, c0:512],
                    kT_sb[:, tbk, (kt % 4) * 128:(kt % 4 + 1) * 128],
                    qT_sb[:, h, b * 4 + qtg, c0:512],
                    start=True, stop=True)
                if causal and kt // 4 == qtg:
                    nc.vector.tensor_add(sps[:, c0:c0 + 128],
                                         sps[:, c0:c0 + 128], masktri_sb[:])
                elif mask_mode == "general":
                    mt = mpool.tile([128, 512], F32)
                    nc.gpsimd.dma_start(
                        out=mt[:],
                        in_=mask_e.ap()[kt * 128:(kt + 1) * 128,
                                        qtg * 512:(qtg + 1) * 512])
                    nc.vector.tensor_add(sps[:], sps[:], mt[:])
                bias = biasB_sb[:] if mask_mode == "general" else 0.0
                nc.scalar.activation(eT[:, kt, c0:512], sps[:, c0:512],
                                     AF.Exp, bias=bias, scale=1.0)

            LA = 5
            for kt in range(min(LA, nkt)):
                s_mm(kt)
            for kt in range(nkt):
                if kt + LA < nkt:
                    s_mm(kt + LA)
                c0 = col0(kt)
                nc.tensor.matmul(otps[:, c0:512], v_sb[:, b * 16 + kt, :],
                                 eT[:, kt, c0:512], start=(kt == 0),
                                 stop=(kt == nkt - 1))
                nc.tensor.matmul(rps[:, c0:512], ones_sb[:],
                                 eT[:, kt, c0:512], start=(kt == 0),
                                 stop=(kt == nkt - 1))

            osb = opool.tile([128, 512], BF16, name="osb", tag="osb")
            nc.vector.tensor_copy(osb[:], otps[:])
            rsb = opool.tile([1, 512], F32, name="rsb", tag="rsb")
            nc.vector.tensor_copy(rsb[:], rps[0:1, :])
            tgt = a2a_in1 if h < 2 else a2a_in2
            hl = h % 2
            nc.sync.dma_start(out=tgt[d, hl * 128:(hl + 1) * 128, :],
                              in_=osb[:])
            nc.sync.dma_start(out=tgt[d, 256 + hl * 2:258 + hl * 2, :],
                              in_=rsb[:].bitcast(BF16))

        for h in range(QH):
            for b in range(B):
                for qtg in range(4):
                    attn_block(h, b, qtg)
            if h == 1:
                # gpsimd queue is otherwise empty in P2, so the trigger
                # waiting at its head costs nothing; completes during
                # head-2/3 compute and the receive below hides too
                nc.gpsimd.collective_compute(
                    "AllToAll", OP.bypass,
                    replica_groups=[list(range(NCORES))],
                    ins=[a2a_in1.opt()], outs=[a2a_out1.opt()])

        # group-1 o^T + rowsums land while heads 2/3 still compute
        for s in range(NCORES):
            for hl in range(2):
                nc.gpsimd.dma_start(
                    out=oT_sb[:, s * QH + hl, :],
                    in_=a2a_out1[s, hl * 128:(hl + 1) * 128, :])
                nc.gpsimd.dma_start(
                    out=rraw[s * 2 + hl:s * 2 + hl + 1, :],
                    in_=a2a_out1[s, 256 + hl * 2:258 + hl * 2, :]
                    .bitcast(F32))
        nc.vector.reciprocal(rinv[0:16, :], rraw[0:16, :])
        nc.sync.dma_start(out=rdram[0], in_=rinv[0:16, :])
        # broadcast 1/r over partitions via stride-0 DRAM reads, then
        # normalize group-1 o^T in place
        for s in range(NCORES):
            for hl in range(2):
                hg = s * QH + hl
                rbt = rbt_tile()
                nc.gpsimd.dma_start(
                    out=rbt[:],
                    in_=bass.AP(tensor=rdram.tensor,
                                offset=rdram.offset + (s * 2 + hl) * 512,
                                ap=[[0, 128], [1, 512]]))
                # gpsimd: the vector queue is saturated with P2 mask adds
                nc.gpsimd.tensor_mul(oT_sb[:, hg, :], oT_sb[:, hg, :],
                                     rbt[:])

        # prefetch the first 8 pass-1 Wo tiles (dc=0) on the idle gpsimd
        # queue so the GEMM is fed the moment P2's last exp drains
        grp1_order = [s * QH + g for s in range(NCORES) for g in (0, 1)]
        for j in range(8):
            hg = grp1_order[j]
            nc.gpsimd.dma_start(
                out=wot_pre[:, j, :],
                in_=Wo_e.ap()[hg * 128:(hg + 1) * 128, 0:512])

        nc.gpsimd.collective_compute(
            "AllToAll", OP.bypass,
            replica_groups=[list(range(NCORES))],
            ins=[a2a_in2.opt()], outs=[a2a_out2.opt()])

    # ---------------- Phase 3: Wo GEMM on own token slice ----------------
    with tc.tile_pool(name="wo_acc", bufs=1) as accpool, \
         tc.tile_pool(name="wo_w", bufs=20) as wopool, \
         tc.tile_pool(name="wo_out", bufs=6) as outpool, \
         tc.tile_pool(name="wo_ps", bufs=2, space="PSUM") as wopsum:

        for s in range(NCORES):
            for hl in range(2):
                nc.gpsimd.dma_start(
                    out=oT_sb[:, s * QH + 2 + hl, :],
                    in_=a2a_out2[s, hl * 128:(hl + 1) * 128, :])
                nc.gpsimd.dma_start(
                    out=rraw[32 + s * 2 + hl:32 + s * 2 + hl + 1, :],
                    in_=a2a_out2[s, 256 + hl * 2:258 + hl * 2, :]
                    .bitcast(F32))
        nc.vector.reciprocal(rinv[32:48, :], rraw[32:48, :])
        nc.sync.dma_start(out=rdram[1], in_=rinv[32:48, :])
        for s in range(NCORES):
            for hl in range(2):
                hg = s * QH + 2 + hl
                rbt = rbt_tile()
                dq = (nc.gpsimd, nc.scalar, nc.sync)[(s * 2 + hl) % 3]
                dq.dma_start(
                    out=rbt[:],
                    in_=bass.AP(tensor=rdram.tensor,
                                offset=rdram.offset + (16 + s * 2 + hl) * 512,
                                ap=[[0, 128], [1, 512]]))
                nc.vector.tensor_mul(oT_sb[:, hg, :], oT_sb[:, hg, :],
                                     rbt[:])

        grp1 = [s * QH + g for s in range(NCORES) for g in (0, 1)]
        grp2 = [s * QH + 2 + g for s in range(NCORES) for g in (0, 1)]
        accs = []
        # pass 1: group-1 contraction only -> SBUF accumulators, giving
        # the second AllToAll a full half-GEMM to hide under
        for dc in range(8):
            pso = wopsum.tile([128, 4, 512], F32, name="pso", tag="pso")
            for ci, hg in enumerate(grp1):
                if dc == 0 and ci < 8:
                    wv = wot_pre[:, ci, :]
                else:
                    wot = wopool.tile([128, 512], BF16)
                    dq = nc.scalar if ci % 2 == 0 else nc.sync
                    dq.dma_start(out=wot[:],
                                 in_=Wo_e.ap()[hg * 128:(hg + 1) * 128,
                                               dc * 512:(dc + 1) * 512])
                    wv = wot[:]
                for t in range(4):
                    nc.tensor.matmul(pso[:, t, :],
                                     oT_sb[:, hg, t * 128:(t + 1) * 128],
                                     wv, start=(ci == 0), stop=(ci == 15))
            acc = accpool.tile([128, 4, 512], F32, name=f"acc{dc}",
                               tag=f"acc{dc}")
            accs.append(acc)
            for t in range(4):
                nc.vector.tensor_copy(acc[:, t, :], pso[:, t, :])
        # pass 2: group-2 contraction + partial sums -> output
        for dc in range(8):
            pso = wopsum.tile([128, 4, 512], F32, name="pso", tag="pso")
            for ci, hg in enumerate(grp2):
                wot = wopool.tile([128, 512], BF16)
                dq = (nc.gpsimd, nc.scalar, nc.sync)[ci % 3]
                dq.dma_start(out=wot[:],
                             in_=Wo_e.ap()[hg * 128:(hg + 1) * 128,
                                           dc * 512:(dc + 1) * 512])
                for t in range(4):
                    nc.tensor.matmul(pso[:, t, :],
                                     oT_sb[:, hg, t * 128:(t + 1) * 128],
                                     wot[:], start=(ci == 0), stop=(ci == 15))
            for t in range(4):
                osb = outpool.tile([128, 512], F32)
                nc.vector.tensor_add(osb[:], accs[dc][:, t, :], pso[:, t, :])
                nc.sync.dma_start(
                    out=out_e.ap()[t * 128:(t + 1) * 128,
                                   dc * 512:(dc + 1) * 512],
                    in_=osb[:])


_NC_CACHE = {}


def _get_nc(mask_mode):
    if mask_mode not in _NC_CACHE:
        _NC_CACHE[mask_mode] = _build(mask_mode)
    return _NC_CACHE[mask_mode]


def _estimate_score_bound(x, Wq, Wk, fc, fs):
    """Sampled upper estimate of max |q.k/sqrt(hd)| after RoPE."""
    rng = np.random.default_rng(12345)
    x2 = x.reshape(TOK, D)
    rq = rng.choice(TOK, 192, replace=False)
    rk = rng.choice(TOK, 192, replace=False)
    q = (x2[rq] @ Wq).reshape(192, NH, HD)
    k = (x2[rk] @ Wk).reshape(192, NKV, HD)

    def rope(t, pos):
        c, s = fc[pos % S], fs[pos % S]
        tr, ti = t[..., 0::2], t[..., 1::2]
        out = np.empty_like(t)
        out[..., 0::2] = tr * c[:, None, :] - ti * s[:, None, :]
        out[..., 1::2] = tr * s[:, None, :] + ti * c[:, None, :]
        return out

    q = rope(q, rq)
    k = rope(k, rk)
    qg = q.reshape(192, NKV, NH // NKV, HD)
    sc = np.einsum('qgnd,kgd->gnqk', qg, k) / np.float32(math.sqrt(HD))
    return float(np.abs(sc).max())


def _rope_tables(freqs_cos, freqs_sin, scale):
    """[128, S] transposed tables: c duplicated on partition pairs,
    s with -sin on even / +sin on odd partitions."""
    c = np.empty((128, S), np.float32)
    s = np.empty((128, S), np.float32)
    c[0::2] = c[1::2] = freqs_cos.T * scale
    s[0::2] = -freqs_sin.T * scale
    s[1::2] = freqs_sin.T * scale
    return np.ascontiguousarray(c), np.ascontiguousarray(s)


def kernel(x, Wq, Wk, Wv, Wo, freqs_cos, freqs_sin, mask, start_pos=0,
           _want_trace=False):
    x = np.asarray(x, dtype=np.float32)
    mask = np.asarray(mask, dtype=np.float32)
    freqs_cos = np.asarray(freqs_cos, dtype=np.float32)
    freqs_sin = np.asarray(freqs_sin, dtype=np.float32)

    if not mask.any():
        mask_mode = "zeros"
    else:
        canon = np.where(np.tril(np.ones((S, S), bool)), 0.0,
                         np.float32(NEG_INF)).astype(np.float32)
        mask_mode = "causal" if np.array_equal(mask, canon) else "general"

    xT = np.ascontiguousarray(x.reshape(TOK, D).T.astype(ml_dtypes.bfloat16))
    scale = np.float32(1.0 / math.sqrt(HD))
    cqT, sqT = _rope_tables(freqs_cos, freqs_sin, scale)
    ckT, skT = _rope_tables(freqs_cos, freqs_sin, np.float32(1.0))
    Wo_bf = np.ascontiguousarray(np.asarray(Wo, np.float32)
                                 .astype(ml_dtypes.bfloat16))
    if mask_mode == "causal":
        # [128,128] upper-triangle additive mask in transposed layout
        p = np.arange(128)[:, None]
        q = np.arange(128)[None, :]
        masktri = np.where(p <= q, 0.0, NEG_INF).astype(np.float32)
        masktri = np.ascontiguousarray(masktri)

    in_maps = []
    for c in range(NCORES):
        m = {
            "xT": xT,
            "Wqkv": np.ascontiguousarray(np.concatenate(
                [Wq[:, c * QH * HD:(c + 1) * QH * HD],
                 Wk[:, c * HD:(c + 1) * HD],
                 Wv[:, c * HD:(c + 1) * HD]],
                axis=1).astype(ml_dtypes.bfloat16)),
            "Wo": Wo_bf,
            "cqT": cqT, "sqT": sqT, "ckT": ckT, "skT": skT,
        }
        if mask_mode == "causal":
            m["masktri"] = masktri
        elif mask_mode == "general":
            m["maskT"] = np.ascontiguousarray(mask.T)
            bound = _estimate_score_bound(x, Wq, Wk, freqs_cos, freqs_sin)
            bmax = float(np.max(mask[np.isfinite(mask)])) if \
                np.isfinite(mask).any() else 0.0
            m["biasB"] = np.full((128, 1), -(1.25 * bound + max(bmax, 0.0)),
                                 np.float32)
        in_maps.append(m)

    nc = _get_nc(mask_mode)
    res = run_bass_kernel_spmd(nc, in_maps, list(range(NCORES)),
                               trace=_want_trace)
    out = np.concatenate([res.results[c]["out"] for c in range(NCORES)],
                         axis=0).reshape(B, S, D)
    if _want_trace:
        return out, res
    return out


if __name__ == "__main__":
    rng = np.random.default_rng(0)
    x = rng.standard_normal((B, S, D), dtype=np.float32) * 0.1
    Wq = rng.standard_normal((D, NH * HD), dtype=np.float32) * 0.02
    Wk = rng.standard_normal((D, NKV * HD), dtype=np.float32) * 0.02
    Wv = rng.standard_normal((D, NKV * HD), dtype=np.float32) * 0.02
    Wo = rng.standard_normal((NH * HD, D), dtype=np.float32) * 0.02
    fc = rng.standard_normal((S, 64), dtype=np.float32)
    fs = rng.standard_normal((S, 64), dtype=np.float32)
    mask = np.where(np.tril(np.ones((S, S), bool)), 0.0,
                    np.float32(NEG_INF)).astype(np.float32)
    out = kernel(x, Wq, Wk, Wv, Wo, fc, fs, mask, 0)
    print("out", out.shape, out.dtype, np.abs(out).mean())
